# revision 25
# baseline (speedup 1.0000x reference)
"""Trainium2 Bass kernel for nn_CenMoEDynamicsModel (MoE routing), v2.

Contract: kernel(**inputs) takes FULL unsharded numpy inputs and returns the
FULL [64, 2048, 128] f32 output. Data-parallel over B across 8 NeuronCores for
routing + combine; expert-parallel (2 experts/core) for the MLP with two tiny
AllToAll exchanges (xin out, expert_outputs back).

Math (per batch b):
  x = [z|a]                       [N, D]     D = 192
  w = x @ phi                     [N, E]     E = 16
  dispatch = softmax_n(w); xin = dispatch^T @ x          [E, D]
  h = mish(LN(xin@W1+b1)); h = mish(LN(h@W2+b2)); EO = h@W3+b3   [E, DZ]
  combine = softmax_e(w); out = combine @ EO             [N, DZ]

Key design points (vs v1 baseline):
  - everything bf16 on the PE (1 cyc/row always; f32r pays 4 cyc/row for
    moving dims < 256) and half the HBM/DMA bytes.
  - token-major routing: out[128tok, E] streams only E=16 rows per matmul
    (vs 512 for e-major), and exp() writes the token-major ec tiles the xin
    matmul needs directly. e-major expCT (combine lhsT) comes from 16 cheap
    PE transposes per batch.
  - combine softmax denominators reduced from ec tiles in phase A (DVE),
    so combine is matmul + one scale/evacuation op per tile, no ones column.
  - expert weights sharded: each core loads 2 experts (1.7 MB bf16 vs 27 MB
    f32 for all 16). xin packs are exchanged with AllToAll ([8,193,16] bf16),
    expert outputs return with a second AllToAll ([8,16,128] bf16).
  - MLP activations packed [2 experts x 64 batches, H] = full 128 partitions;
    PE writes both experts into one PSUM tile (offsets 0/64). Inter-layer
    transposes use the DMA crossbar (dma_start_transpose), not the PE.
  - single top-level SBUF pool scope => weight prefetch DMAs (gpsimd queue)
    overlap phase A input streaming (sync queue); no inter-phase SBUF
    anti-dependency barriers.

ones-column tricks kept from v1: 4 ones columns appended to x give the
dispatch denominator s_e as xin row D; W1 is bias-augmented so scaling L1 rows
by r = 1/s normalizes dispatch and bias exactly.
"""

import sys

import numpy as np

sys.path.insert(0, "/opt/trn_rl_repo")

from contextlib import ExitStack

import concourse.bass as bass
import concourse.tile as tile
from concourse import mybir

F32 = mybir.dt.float32
BF16 = mybir.dt.bfloat16
AF = mybir.ActivationFunctionType

LN_EPS = 1e-5
NCORES = 8


def _split_drain_and_barrier(self, tick_clock, wait_clock):
    """Replacement for TileContext._drain_and_barrier.

    The stock version attaches every outstanding semaphore wait to ONE tail
    Drain instruction; this walrus build's codegen rejects Drains with more
    than a couple of sync waits ("Too many sync wait commands"). Emit one
    single-wait Drain per logical proc instead (the SP queue executes them in
    order, so the final bare drain still happens after everything finished).
    """
    from concourse.vector_clock import ScopedClock, VectorClock

    nc = self.nc
    gc = tick_clock.global_clock
    n = len(gc)
    for i in range(n):
        t = gc[i]
        if t <= 0:
            continue
        v = VectorClock([0] * n)
        v.require_at_least(i, t)
        d = nc.sync.drain()
        wait_clock.add_sem_waits(d.ins, ScopedClock({None: v}))
    nc.sync.drain()
    nc.all_engine_barrier()
    assert self.sems is not None
    popped = nc._tile_sem_poison_stack.pop()
    assert popped is self._sem_poison
    nc.clear_and_free_semaphores(list(self.sems.allocated().values()))
    nc.all_engine_barrier()


tile.TileContext._drain_and_barrier = _split_drain_and_barrier

# This walrus build rejects instructions carrying more than a couple of sync
# waits ("Too many sync wait commands" in CoreV3 codegen), while Tile freely
# attaches 3+. Split excess waits onto NoOp carrier instructions (same engine
# queue, executed in order => semantics preserved) at BIR-serialization time.
_MAX_WAITS = 1


def _split_waits_json(bir: bytes) -> bytes:
    import orjson

    m = orjson.loads(bir)
    changed = False
    ctr = 0
    for f in m.get("functions", []):
        for b in f.get("blocks", []):
            out = []
            for i in b.get("instructions", []):
                si = i.get("sync_info")
                ow = (si or {}).get("on_wait") or []
                if len(ow) > _MAX_WAITS:
                    head = ow[: -_MAX_WAITS]
                    for j in range(0, len(head), _MAX_WAITS):
                        ctr += 1
                        out.append(
                            {
                                "debug": i.get("debug", 0),
                                "engine": i["engine"],
                                "ins": [],
                                "outs": [],
                                "name": f"{i['name']}-wsplit{ctr}",
                                "opcode": "NoOp",
                                "sync_info": {
                                    "on_wait": head[j : j + _MAX_WAITS],
                                    "on_update": [],
                                },
                            }
                        )
                    si["on_wait"] = ow[-_MAX_WAITS:]
                    changed = True
                out.append(i)
            b["instructions"] = out
    return orjson.dumps(m) if changed else bir


_orig_to_json_bytes = bass.Bass.to_json_bytes


def _patched_to_json_bytes(self):
    return _split_waits_json(_orig_to_json_bytes(self))


bass.Bass.to_json_bytes = _patched_to_json_bytes


def build_nc(BC, N, DZ, DA, E, H1, H2, has_b2, has_b3, has_g1, has_g2):
    """Build the per-core Bass program (SPMD, rank-free).

    BC batches per core. EL = E // NCORES local experts.
    """
    D = DZ + DA  # 192
    NT = N // 128  # 16 token tiles per batch
    EL = E // NCORES  # 2
    XA = D + 4  # x padded with 4 ones columns (col D = s_e trick)
    C1 = H1 // 128
    C2 = H2 // 128
    ROWS = EL * NCORES * BC  # 128 MLP pack rows = (el, c, b)
    assert DZ == 128 and E == 16 and BC == 8 and EL == 2 and ROWS == 128

    nc = bass.Bass(num_devices=NCORES)
    import os as _os
    for _ in range(int(_os.environ.get("K_SALT", "0"))):
        nc.sync.nop()

    xa = nc.dram_tensor("xa", [BC, 128, NT, XA], BF16, kind="ExternalInput")
    zTs = nc.dram_tensor("zTs", [BC, 128, NT, 128], BF16, kind="ExternalInput")
    aTs = nc.dram_tensor("aTs", [BC, DA, NT, 128], BF16, kind="ExternalInput")
    phi_d = nc.dram_tensor("phi_d", [D, E], BF16, kind="ExternalInput")
    w1h_d = nc.dram_tensor("w1h_d", [EL, 128, H1], BF16, kind="ExternalInput")
    w1l_d = nc.dram_tensor("w1l_d", [EL, D - 128 + 1, H1], BF16, kind="ExternalInput")
    w2_d = nc.dram_tensor("w2_d", [EL, C1, 128, H2], BF16, kind="ExternalInput")
    w3_d = nc.dram_tensor("w3_d", [EL, C2, 128, DZ], BF16, kind="ExternalInput")
    identb_d = nc.dram_tensor("identb_d", [128, 128], BF16, kind="ExternalInput")
    if has_b2:
        b2_d = nc.dram_tensor("b2_d", [1, EL * H2], BF16, kind="ExternalInput")
        ones_d = nc.dram_tensor("ones_d", [1, ROWS], BF16, kind="ExternalInput")
    if has_b3:
        b3_d = nc.dram_tensor("b3_d", [1, EL * DZ], BF16, kind="ExternalInput")
    if has_g1:
        g1_d = nc.dram_tensor("g1_d", [ROWS, H1], F32, kind="ExternalInput")
        be1_d = nc.dram_tensor("be1_d", [ROWS, H1], F32, kind="ExternalInput")
    if has_g2:
        g2_d = nc.dram_tensor("g2_d", [ROWS, H2], F32, kind="ExternalInput")
        be2_d = nc.dram_tensor("be2_d", [ROWS, H2], F32, kind="ExternalInput")
    out = nc.dram_tensor("out", [BC, 128, NT, DZ], F32, kind="ExternalOutput")

    LO = D - 128 + 1  # 65 rows: a-features 128..191 plus the s row

    with tile.TileContext(nc) as tc, ExitStack() as ctx:
        perm = ctx.enter_context(tc.tile_pool(name="perm", bufs=1))
        pa = ctx.enter_context(tc.tile_pool(name="pa", bufs=2))
        pec = ctx.enter_context(tc.tile_pool(name="pec", bufs=4))
        pm = ctx.enter_context(tc.tile_pool(name="pm", bufs=1))
        pst = ctx.enter_context(tc.tile_pool(name="pst", bufs=2))
        posb = ctx.enter_context(tc.tile_pool(name="posb", bufs=2))
        dram = ctx.enter_context(tc.tile_pool(name="dram", bufs=1, space="DRAM"))

        # ---------------- weight / constant prefetch (gpsimd queue) --------
        identb = perm.tile([128, 128], BF16)
        nc.gpsimd.dma_start(identb[:], identb_d[:, :])
        phi_hi = perm.tile([128, E], BF16)
        phi_lo = perm.tile([DA, E], BF16)
        nc.gpsimd.dma_start(phi_hi[:], phi_d[0:128, :])
        nc.gpsimd.dma_start(phi_lo[:], phi_d[128:D, :])
        w1h_sb = perm.tile([128, EL * H1], BF16)
        w1l_sb = perm.tile([LO, EL * H1], BF16)
        w2_sb = perm.tile([128, EL * C1 * H2], BF16)
        w3_sb = perm.tile([128, EL * C2 * DZ], BF16)
        nc.gpsimd.dma_start(
            w1h_sb[:].rearrange("p (el h) -> p el h", el=EL), w1h_d[:, :, :].rearrange("el p h -> p el h")
        )
        nc.gpsimd.dma_start(
            w1l_sb[:].rearrange("p (el h) -> p el h", el=EL), w1l_d[:, :, :].rearrange("el p h -> p el h")
        )
        nc.gpsimd.dma_start(
            w2_sb[:].rearrange("p (el c h) -> p el c h", el=EL, c=C1),
            w2_d[:, :, :, :].rearrange("el c p h -> p el c h"),
        )
        nc.gpsimd.dma_start(
            w3_sb[:].rearrange("p (el c h) -> p el c h", el=EL, c=C2),
            w3_d[:, :, :, :].rearrange("el c p h -> p el c h"),
        )
        if has_b2:
            b2_sb = perm.tile([1, EL * H2], BF16)
            nc.gpsimd.dma_start(b2_sb[:], b2_d[:, :])
            ones_sb = perm.tile([1, ROWS], BF16)
            nc.gpsimd.dma_start(ones_sb[:], ones_d[:, :])
        if has_b3:
            b3_sb = perm.tile([1, EL * DZ], BF16)
            nc.gpsimd.dma_start(b3_sb[:], b3_d[:, :])
        g1_sb = be1_sb = g2_sb = be2_sb = None
        if has_g1:
            g1_sb = perm.tile([ROWS, H1], F32)
            be1_sb = perm.tile([ROWS, H1], F32)
            nc.gpsimd.dma_start(g1_sb[:], g1_d[:, :])
            nc.gpsimd.dma_start(be1_sb[:], be1_d[:, :])
        if has_g2:
            g2_sb = perm.tile([ROWS, H2], F32)
            be2_sb = perm.tile([ROWS, H2], F32)
            nc.gpsimd.dma_start(g2_sb[:], g2_d[:, :])
            nc.gpsimd.dma_start(be2_sb[:], be2_d[:, :])

        # dummy collective: absorbs the first-collective warmup latency (the
        # first CC on this fabric costs ~15us extra) while phase A computes
        ccw_in = dram.tile([NCORES, 2], F32, name="ccw_in")
        ccw_out = dram.tile([NCORES, 2], F32, name="ccw_out")
        ccw_sb = perm.tile([NCORES, 2], F32)
        nc.gpsimd.memset(ccw_sb[:], 0.0)
        nc.gpsimd.dma_start(ccw_in[:], ccw_sb[:])
        for _ in range(2):
            nc.gpsimd.collective_compute(
                "AllToAll",
                mybir.AluOpType.bypass,
                replica_groups=[list(range(NCORES))],
                ins=[ccw_in[:].opt()],
                outs=[ccw_out[:].opt()],
            )

        # persistent phase-A outputs
        expCT = [perm.tile([E, N], BF16, name=f"expCT{b}") for b in range(BC)]
        rcomb = [perm.tile([128, NT], F32, name=f"rcomb{b}") for b in range(BC)]
        # xin packs split into batch halves so the first AllToAll can launch
        # while phase A still works on batches BC/2..BC-1
        HB = BC // 2
        xin_hi_h = [perm.tile([128, E * HB], BF16, name=f"xinh{h}") for h in range(2)]
        xin_lo_h = [perm.tile([LO, E * HB], BF16, name=f"xinl{h}") for h in range(2)]
        eps_col = perm.tile([128, 1], F32)
        nc.vector.memset(eps_col[:], LN_EPS)

        cc1_in = [
            dram.tile([NCORES, 128 + LO, EL * HB], BF16, name=f"cc1_in{h}")
            for h in range(2)
        ]
        cc1_out = [
            dram.tile([NCORES, 128 + LO, EL * HB], BF16, name=f"cc1_out{h}")
            for h in range(2)
        ]

        def emit_cc1(h):
            nc.gpsimd.dma_start(
                cc1_in[h][:, 0:128, :].rearrange("j d q -> d j q"),
                xin_hi_h[h][:].rearrange("p (j q) -> p j q", j=NCORES),
            )
            nc.gpsimd.dma_start(
                cc1_in[h][:, 128 : 128 + LO, :].rearrange("j d q -> d j q"),
                xin_lo_h[h][:].rearrange("p (j q) -> p j q", j=NCORES),
            )
            nc.gpsimd.collective_compute(
                "AllToAll",
                mybir.AluOpType.bypass,
                replica_groups=[list(range(NCORES))],
                ins=[cc1_in[h][:].opt()],
                outs=[cc1_out[h][:].opt()],
            )

        # ---------------- Phase A: routing + xin (data-parallel) -----------
        with tc.tile_pool(name="ps_wq", bufs=3, space="PSUM") as ps_wq, tc.tile_pool(
            name="ps_xin", bufs=2, space="PSUM"
        ) as ps_xin, tc.tile_pool(
            name="ps_xtr", bufs=1, space="PSUM"
        ) as ps_xtr:
            for b in range(BC):
                zT_sb = pa.tile([128, NT * 128], BF16, tag="zT")
                aT_sb = pa.tile([DA, NT * 128], BF16, tag="aT")
                nc.sync.dma_start(
                    zT_sb[:].rearrange("p (t c) -> p t c", c=128), zTs[b]
                )
                nc.sync.dma_start(
                    aT_sb[:].rearrange("p (t c) -> p t c", c=128), aTs[b]
                )
                x_sb = pa.tile([128, NT * XA], BF16, tag="x")
                xv = x_sb[:].rearrange("p (t c) -> p t c", c=XA)
                nc.sync.dma_start(xv[:, :, :], xa[b])
                zTv = zT_sb[:].rearrange("p (t c) -> p t c", c=128)
                aTv = aT_sb[:].rearrange("p (t c) -> p t c", c=128)

                xinps = ps_xin.tile([E, XA], F32, tag="xin")
                denom = pa.tile([128, NT], F32, tag="denom")
                for g in range(NT // 4):
                    # e-major routing: one [16, 512] chunk covers 4 token
                    # tiles; exp writes the combine lhsT (expCT) directly
                    sl512 = slice(512 * g, 512 * (g + 1))
                    wqe = ps_wq.tile([E, 512], F32, tag="wq")
                    nc.tensor.matmul(
                        wqe[:], phi_hi[:], zT_sb[:, sl512], start=True, stop=False
                    )
                    nc.tensor.matmul(
                        wqe[:], phi_lo[:], aT_sb[:, sl512], start=False, stop=True
                    )
                    nc.scalar.activation(expCT[b][:, sl512], wqe[:], AF.Exp)
                # one crossbar DMA transposes the whole [E, N] expCT into
                # token-major [128, (t, e)] tiles (no PE transposes/copies)
                ec_all = pec.tile([128, NT * E], BF16, tag="ec")
                nc.sync.dma_start_transpose(
                    ec_all[:].rearrange("p (t e) -> p t e", t=NT), expCT[b][:]
                )
                ecv = ec_all[:].rearrange("p (t e) -> p t e", t=NT)
                for t in range(NT):
                    # xin accumulation (contract tokens)
                    nc.tensor.matmul(
                        xinps[:],
                        ecv[:, t, :],
                        xv[:, t, :],
                        start=(t == 0),
                        stop=(t == NT - 1),
                    )
                # combine denominators: one reduce over all 16 tiles
                nc.vector.tensor_reduce(
                    denom[:],
                    ecv[:, :, :],
                    mybir.AxisListType.X,
                    mybir.AluOpType.add,
                )
                # reciprocal of combine denominators (all 16 tiles at once)
                nc.vector.reciprocal(rcomb[b][:], denom[:])
                # xin -> bf16 -> transposed into the [d, (e b-half)] packs
                xin_sb = pec.tile([E, XA], BF16, tag="xin_sb")
                nc.scalar.copy(xin_sb[:], xinps[:])
                h, bb = b // HB, b % HB
                xhv = xin_hi_h[h][:].rearrange("p (e b) -> p e b", b=HB)
                xlv = xin_lo_h[h][:].rearrange("p (e b) -> p e b", b=HB)
                pth = ps_xtr.tile([128, E], BF16, tag="trh")
                nc.tensor.transpose(pth[:], xin_sb[:, 0:128], identb[0:E, 0:E])
                nc.vector.tensor_copy(xhv[:, :, bb], pth[:])
                ptl = ps_xtr.tile([LO + 1, E], BF16, tag="trl")
                nc.tensor.transpose(
                    ptl[:], xin_sb[:, 128 : 128 + LO + 1], identb[0:E, 0:E]
                )
                nc.scalar.copy(xlv[:, :, bb], ptl[0:LO, :])
                if b == HB - 1:
                    emit_cc1(0)

        # xinp cols: (h, el, c, bb) -- MLP rows follow the same order, so the
        # L1 matmuls for half 0 can run while phase A / cc1b still execute
        xinp_hi_h = [
            perm.tile([128, ROWS // 2], BF16, name=f"xph{h}") for h in range(2)
        ]
        xinp_lo_h2 = [
            perm.tile([LO, ROWS // 2], BF16, name=f"xpl{h}") for h in range(2)
        ]

        def emit_cc1_load(h):
            for el in range(EL):
                csl = slice(32 * el, 32 * (el + 1))
                qsl = slice(HB * el, HB * (el + 1))
                eng = nc.gpsimd if el == 0 else nc.sync
                eng2 = nc.gpsimd if el == 0 else nc.scalar
                eng.dma_start(
                    xinp_hi_h[h][:, csl].rearrange("p (c b) -> p c b", c=NCORES),
                    cc1_out[h][:, 0:128, qsl].rearrange("c d b -> d c b"),
                )
                eng2.dma_start(
                    xinp_lo_h2[h][:, csl].rearrange("p (c b) -> p c b", c=NCORES),
                    cc1_out[h][:, 128 : 128 + LO, qsl].rearrange("c d b -> d c b"),
                )
            # r for this half: 1/s from the s row of xinp_lo
            nc.vector.tensor_copy(
                r_row[0:1, 64 * h : 64 * (h + 1)], xinp_lo_h2[h][LO - 1 : LO, :]
            )
            nc.vector.reciprocal(
                r_row[0:1, 64 * h : 64 * (h + 1)], r_row[0:1, 64 * h : 64 * (h + 1)]
            )
            nc.gpsimd.dma_start(
                r_col[64 * h : 64 * (h + 1), :], r_row[0:1, 64 * h : 64 * (h + 1)]
            )

        # r = 1/s per MLP row (h, el, c, bb); s sits in xinp_lo row LO-1
        r_row = perm.tile([1, ROWS], F32)
        r_col = perm.tile([128, 1], F32)

        # ---------------- MLP (expert-parallel, rows = (el, c, b)) ---------
        def ln_mish(hs, H, gr, ber):
            """LayerNorm + mish of SBUF [128, H] f32 -> bf16.

            Uses only Exp/Ln/Square activations (one act table, no reload):
              rstd = exp(-0.5 * ln(var + eps))
              mish(x) = x * tanh(ln(u)), u = 1 + e^x
                      = x * (1 - 2 * exp(-ln(u^2 + 1)))
            """
            stats = pm.tile([128, 6], F32, tag="stats")
            nc.vector.bn_stats(stats[:], hs)
            mv = pm.tile([128, 2], F32, tag="mv")
            nc.vector.bn_aggr(mv[:], stats[:])
            lnv = pm.tile([128, 1], F32, tag="lnv")
            nc.scalar.activation(lnv[:], mv[:, 1:2], AF.Ln, bias=eps_col[:])
            rstd = pm.tile([128, 1], F32, tag="rstd")
            nc.scalar.activation(rstd[:], lnv[:], AF.Exp, scale=-0.5)
            xn = pm.tile([128, H], F32, tag="xn")
            nc.vector.tensor_scalar(
                xn[:], hs, mv[:, 0:1], rstd[:],
                mybir.AluOpType.subtract, mybir.AluOpType.mult,
            )
            if gr is not None:
                xg = pm.tile([128, H], F32, tag="xg")
                nc.vector.tensor_mul(xg[:], xn[:], gr)
                xn = pm.tile([128, H], F32, tag="xb")
                nc.vector.tensor_add(xn[:], xg[:], ber)
            ex = pm.tile([128, H], F32, tag="ex")
            nc.scalar.activation(ex[:], xn[:], AF.Exp)
            sq2 = pm.tile([128, H], F32, tag="sq2")
            nc.scalar.activation(sq2[:], ex[:], AF.Square, bias=1.0)
            ln2 = pm.tile([128, H], F32, tag="ln2")
            nc.scalar.activation(ln2[:], sq2[:], AF.Ln, bias=1.0)
            wv = pm.tile([128, H], F32, tag="wv")
            nc.scalar.activation(wv[:], ln2[:], AF.Exp, scale=-1.0)
            m = pm.tile([128, H], F32, tag="m")
            nc.vector.tensor_scalar(
                m[:], wv[:], -2.0, 1.0, mybir.AluOpType.mult, mybir.AluOpType.add
            )
            hm = pm.tile([128, H], BF16, tag="hm")
            nc.vector.tensor_mul(hm[:], xn[:], m[:])
            return hm

        w1h_v = w1h_sb[:].rearrange("p (el h) -> p el h", el=EL)
        w1l_v = w1l_sb[:].rearrange("p (el h) -> p el h", el=EL)
        w2_v = w2_sb[:].rearrange("p (el c h) -> p el c h", el=EL, c=C1)
        w3_v = w3_sb[:].rearrange("p (el c h) -> p el c h", el=EL, c=C2)

        with tc.tile_pool(name="ps_mlp", bufs=2, space="PSUM") as ps_mlp, tc.tile_pool(
            name="ps_eo", bufs=1, space="PSUM"
        ) as ps_eo:
            h1ps = ps_mlp.tile([128, H1], F32, tag="h12")

            def emit_l1(h):
                for el in range(EL):
                    osl = slice(64 * h + 32 * el, 64 * h + 32 * (el + 1))
                    csl = slice(32 * el, 32 * (el + 1))
                    tp = (0, 64 * h + 32 * el)
                    nc.tensor.matmul(
                        h1ps[osl, :],
                        xinp_hi_h[h][:, csl],
                        w1h_v[:, el, :],
                        start=True,
                        stop=False,
                        tile_position=tp,
                    )
                    nc.tensor.matmul(
                        h1ps[osl, :],
                        xinp_lo_h2[h][:, csl],
                        w1l_v[:, el, :],
                        start=False,
                        stop=True,
                        tile_position=tp,
                    )

            # half 0: loads + L1 while cc1b is still in flight
            emit_cc1_load(0)
            emit_l1(0)
            emit_cc1(1)
            emit_cc1_load(1)
            emit_l1(1)
            h1s = pm.tile([128, H1], F32, tag="h1s")
            nc.vector.tensor_scalar_mul(h1s[:], h1ps[:], r_col[:])
            h1m = ln_mish(h1s[:], H1, g1_sb[:] if has_g1 else None, be1_sb[:] if has_g1 else None)
            h1T = pm.tile([128, C1 * 128], BF16, tag="h1T")
            nc.sync.dma_start_transpose(
                h1T[:].rearrange("p (c m) -> p c m", c=C1), h1m[:]
            )

            h2ps = ps_mlp.tile([128, H2], F32, tag="h12")
            for h in range(2):
                for el in range(EL):
                    osl = slice(64 * h + 32 * el, 64 * h + 32 * (el + 1))
                    tp = (0, 64 * h + 32 * el)
                    for c in range(C1):
                        nc.tensor.matmul(
                            h2ps[osl, :],
                            h1T[:, c * 128 + 64 * h + 32 * el : c * 128 + 64 * h + 32 * (el + 1)],
                            w2_v[:, el, c, :],
                            start=(c == 0),
                            stop=(c == C1 - 1 and not has_b2),
                            tile_position=tp,
                        )
                    if has_b2:
                        nc.tensor.matmul(
                            h2ps[osl, :],
                            ones_sb[0:1, 0:32],
                            b2_sb[0:1, el * H2 : (el + 1) * H2],
                            start=False,
                            stop=True,
                            tile_position=tp,
                        )
            h2s = pm.tile([128, H2], F32, tag="h2s")
            nc.vector.tensor_copy(h2s[:], h2ps[:])
            h2m = ln_mish(h2s[:], H2, g2_sb[:] if has_g2 else None, be2_sb[:] if has_g2 else None)
            h2T = pm.tile([128, C2 * 128], BF16, tag="h2T")
            nc.sync.dma_start_transpose(
                h2T[:].rearrange("p (c m) -> p c m", c=C2), h2m[:]
            )

            eops = ps_eo.tile([128, DZ], F32, tag="eo")
            for h in range(2):
                for el in range(EL):
                    osl = slice(64 * h + 32 * el, 64 * h + 32 * (el + 1))
                    tp = (0, 64 * h + 32 * el)
                    for c in range(C2):
                        nc.tensor.matmul(
                            eops[osl, :],
                            h2T[:, c * 128 + 64 * h + 32 * el : c * 128 + 64 * h + 32 * (el + 1)],
                            w3_v[:, el, c, :],
                            start=(c == 0),
                            stop=(c == C2 - 1 and not has_b3),
                            tile_position=tp,
                        )
                    if has_b3:
                        nc.tensor.matmul(
                            eops[osl, :],
                            ones_sb[0:1, 0:32],
                            b3_sb[0:1, el * DZ : (el + 1) * DZ],
                            start=False,
                            stop=True,
                            tile_position=tp,
                        )
            eo_sb = pm.tile([128, DZ], BF16, tag="eo_sb")
            nc.vector.tensor_copy(eo_sb[:], eops[:])

        # ---------------- AllToAll 2: expert outputs back ------------------
        cc2_in = dram.tile([NCORES, EL, BC, DZ], BF16)
        cc2_out = dram.tile([NCORES, EL, BC, DZ], BF16)
        for h in range(2):
            for el in range(EL):
                eng = (nc.gpsimd, nc.scalar, nc.sync, nc.gpsimd)[2 * h + el]
                eng.dma_start(
                    cc2_in[:, el, HB * h : HB * (h + 1), :],
                    eo_sb[64 * h + 32 * el : 64 * h + 32 * (el + 1), :],
                )
        nc.gpsimd.collective_compute(
            "AllToAll",
            mybir.AluOpType.bypass,
            replica_groups=[list(range(NCORES))],
            ins=[cc2_in[:].opt()],
            outs=[cc2_out[:].opt()],
        )
        eo_b = [perm.tile([E, DZ], BF16, name=f"eo{b}") for b in range(BC)]
        for b in range(BC):
            eng = (nc.gpsimd, nc.scalar, nc.sync)[b % 3]
            eng.dma_start(eo_b[b][:], cc2_out[:, :, b, :])

        # ---------------- Combine (data-parallel) --------------------------
        with tc.tile_pool(name="ps_cmb", bufs=4, space="PSUM") as ps_cmb:
            for b in range(BC):
                osb = posb.tile([128, NT * DZ], F32, tag="osb")
                ov = osb[:].rearrange("p (t d) -> p t d", d=DZ)
                for t in range(NT):
                    cps = ps_cmb.tile([128, DZ], F32, tag="c")
                    nc.tensor.matmul(
                        cps[:],
                        expCT[b][:, 128 * t : 128 * (t + 1)],
                        eo_b[b][:],
                        start=True,
                        stop=True,
                    )
                    eng = (nc.vector, nc.scalar)[t % 2]
                    if eng is nc.scalar:
                        eng.mul(ov[:, t, :], cps[:], rcomb[b][:, t : t + 1])
                    else:
                        eng.tensor_scalar_mul(
                            ov[:, t, :], cps[:], rcomb[b][:, t : t + 1]
                        )
                eng = nc.gpsimd if b % 2 == 0 else nc.sync
                eng.dma_start(out[b], ov[:, :, :])
    return nc


# ---------------------------------------------------------------------------
# Host wrapper
# ---------------------------------------------------------------------------

_CACHE = {}


def _get_nc(key, *args):
    if key not in _CACHE:
        _CACHE[key] = build_nc(*args)
    return _CACHE[key]


def _prepare(z, a, phi, W1, b1, g1, be1, W2, b2, g2, be2, W3, b3):
    """Build (cached) the Bass program and per-core input maps."""
    import ml_dtypes

    BF = ml_dtypes.bfloat16

    z = np.asarray(z, np.float32)
    a = np.asarray(a, np.float32)
    phi = np.asarray(phi, np.float32)
    W1 = np.asarray(W1, np.float32)
    b1 = np.asarray(b1, np.float32)
    g1 = np.asarray(g1, np.float32)
    be1 = np.asarray(be1, np.float32)
    W2 = np.asarray(W2, np.float32)
    b2 = np.asarray(b2, np.float32)
    g2 = np.asarray(g2, np.float32)
    be2 = np.asarray(be2, np.float32)
    W3 = np.asarray(W3, np.float32)
    b3 = np.asarray(b3, np.float32)

    B, N, DZ = z.shape
    DA = a.shape[2]
    D = DZ + DA
    E = W1.shape[0]
    H1 = W1.shape[2]
    H2 = W2.shape[2]
    BC = B // NCORES
    EL = E // NCORES
    NT = N // 128
    XA = D + 4

    has_b2 = bool(np.any(b2))
    has_b3 = bool(np.any(b3))
    has_g1 = not (np.all(g1 == 1.0) and np.all(be1 == 0.0))
    has_g2 = not (np.all(g2 == 1.0) and np.all(be2 == 0.0))

    key = (BC, N, DZ, DA, E, H1, H2, has_b2, has_b3, has_g1, has_g2)
    nc = _get_nc(key, *key)

    # x native, padded with ones: [B, N, XA] -> [B, 128, NT, XA]
    xf = np.empty((B, N, XA), np.float32)
    xf[:, :, 0:DZ] = z
    xf[:, :, DZ:D] = a
    xf[:, :, D:XA] = 1.0
    xa_all = np.ascontiguousarray(
        xf.reshape(B, 128, NT, XA)
    ).astype(BF)  # token n = p*NT + t
    # x transposed, tile-shuffled: [B, D, NT, 128] with col (t, p)
    xT = np.concatenate([z, a], axis=-1).transpose(0, 2, 1)  # [B, D, N]
    xTs = xT.reshape(B, D, 128, NT).transpose(0, 1, 3, 2)  # [B, D, NT, 128]
    zTs_all = np.ascontiguousarray(xTs[:, 0:128]).astype(BF)
    aTs_all = np.ascontiguousarray(xTs[:, 128:D]).astype(BF)

    phi2 = np.ascontiguousarray(phi.reshape(D, -1)).astype(BF)
    w1aug = np.concatenate([W1, b1[:, None, :]], axis=1)  # [E, D+1, H1]
    identb = np.eye(128, dtype=np.float32).astype(BF)

    in_maps = []
    for i in range(NCORES):
        es = slice(i * EL, (i + 1) * EL)
        m = {
            "xa": xa_all[i * BC : (i + 1) * BC],
            "zTs": zTs_all[i * BC : (i + 1) * BC],
            "aTs": aTs_all[i * BC : (i + 1) * BC],
            "phi_d": phi2,
            "w1h_d": np.ascontiguousarray(w1aug[es, 0:128]).astype(BF),
            "w1l_d": np.ascontiguousarray(w1aug[es, 128 : D + 1]).astype(BF),
            "w2_d": np.ascontiguousarray(
                W2[es].reshape(EL, H1 // 128, 128, H2)
            ).astype(BF),
            "w3_d": np.ascontiguousarray(
                W3[es].reshape(EL, H2 // 128, 128, DZ)
            ).astype(BF),
            "identb_d": identb,
        }
        if has_b2:
            m["b2_d"] = np.ascontiguousarray(b2[es].reshape(1, -1)).astype(BF)
            m["ones_d"] = np.ones((1, 128), np.float32).astype(BF)
        if has_b3:
            m["b3_d"] = np.ascontiguousarray(b3[es].reshape(1, -1)).astype(BF)
        if has_g1:
            m["g1_d"] = np.ascontiguousarray(np.tile(np.repeat(g1[es], 32, 0), (2, 1)))
            m["be1_d"] = np.ascontiguousarray(np.tile(np.repeat(be1[es], 32, 0), (2, 1)))
        if has_g2:
            m["g2_d"] = np.ascontiguousarray(np.tile(np.repeat(g2[es], 32, 0), (2, 1)))
            m["be2_d"] = np.ascontiguousarray(np.tile(np.repeat(be2[es], 32, 0), (2, 1)))
        in_maps.append(m)
    return nc, in_maps


def kernel(**inputs):
    nc, in_maps = _prepare(**inputs)

    from concourse.bass_utils import run_bass_kernel_spmd

    res = run_bass_kernel_spmd(nc, in_maps, list(range(NCORES)))
    B = len(in_maps) * in_maps[0]["xa"].shape[0]
    outs = []
    for r in res.results:
        o = r["out"]  # [BC, 128, NT, DZ], token n = p*NT + t
        BC, P, NT_, DZ_ = o.shape
        outs.append(o.reshape(BC, P * NT_, DZ_))
    return np.concatenate(outs, axis=0)


# revision 27
# speedup vs baseline: 1.0049x; 1.0049x over previous
"""Trainium2 Bass kernel for nn_CenMoEDynamicsModel (MoE routing), v2.

Contract: kernel(**inputs) takes FULL unsharded numpy inputs and returns the
FULL [64, 2048, 128] f32 output. Data-parallel over B across 8 NeuronCores for
routing + combine; expert-parallel (2 experts/core) for the MLP with two tiny
AllToAll exchanges (xin out, expert_outputs back).

Math (per batch b):
  x = [z|a]                       [N, D]     D = 192
  w = x @ phi                     [N, E]     E = 16
  dispatch = softmax_n(w); xin = dispatch^T @ x          [E, D]
  h = mish(LN(xin@W1+b1)); h = mish(LN(h@W2+b2)); EO = h@W3+b3   [E, DZ]
  combine = softmax_e(w); out = combine @ EO             [N, DZ]

Key design points (vs v1 baseline):
  - everything bf16 on the PE (1 cyc/row always; f32r pays 4 cyc/row for
    moving dims < 256) and half the HBM/DMA bytes.
  - token-major routing: out[128tok, E] streams only E=16 rows per matmul
    (vs 512 for e-major), and exp() writes the token-major ec tiles the xin
    matmul needs directly. e-major expCT (combine lhsT) comes from 16 cheap
    PE transposes per batch.
  - combine softmax denominators reduced from ec tiles in phase A (DVE),
    so combine is matmul + one scale/evacuation op per tile, no ones column.
  - expert weights sharded: each core loads 2 experts (1.7 MB bf16 vs 27 MB
    f32 for all 16). xin packs are exchanged with AllToAll ([8,193,16] bf16),
    expert outputs return with a second AllToAll ([8,16,128] bf16).
  - MLP activations packed [2 experts x 64 batches, H] = full 128 partitions;
    PE writes both experts into one PSUM tile (offsets 0/64). Inter-layer
    transposes use the DMA crossbar (dma_start_transpose), not the PE.
  - single top-level SBUF pool scope => weight prefetch DMAs (gpsimd queue)
    overlap phase A input streaming (sync queue); no inter-phase SBUF
    anti-dependency barriers.

ones-column tricks kept from v1: 4 ones columns appended to x give the
dispatch denominator s_e as xin row D; W1 is bias-augmented so scaling L1 rows
by r = 1/s normalizes dispatch and bias exactly.
"""

import sys

import numpy as np

sys.path.insert(0, "/opt/trn_rl_repo")

from contextlib import ExitStack

import concourse.bass as bass
import concourse.tile as tile
from concourse import mybir

F32 = mybir.dt.float32
BF16 = mybir.dt.bfloat16
AF = mybir.ActivationFunctionType

LN_EPS = 1e-5
NCORES = 8


def _split_drain_and_barrier(self, tick_clock, wait_clock):
    """Replacement for TileContext._drain_and_barrier.

    The stock version attaches every outstanding semaphore wait to ONE tail
    Drain instruction; this walrus build's codegen rejects Drains with more
    than a couple of sync waits ("Too many sync wait commands"). Emit one
    single-wait Drain per logical proc instead (the SP queue executes them in
    order, so the final bare drain still happens after everything finished).
    """
    from concourse.vector_clock import ScopedClock, VectorClock

    nc = self.nc
    gc = tick_clock.global_clock
    n = len(gc)
    for i in range(n):
        t = gc[i]
        if t <= 0:
            continue
        v = VectorClock([0] * n)
        v.require_at_least(i, t)
        d = nc.sync.drain()
        wait_clock.add_sem_waits(d.ins, ScopedClock({None: v}))
    nc.sync.drain()
    nc.all_engine_barrier()
    assert self.sems is not None
    popped = nc._tile_sem_poison_stack.pop()
    assert popped is self._sem_poison
    nc.clear_and_free_semaphores(list(self.sems.allocated().values()))
    nc.all_engine_barrier()


tile.TileContext._drain_and_barrier = _split_drain_and_barrier

# This walrus build rejects instructions carrying more than a couple of sync
# waits ("Too many sync wait commands" in CoreV3 codegen), while Tile freely
# attaches 3+. Split excess waits onto NoOp carrier instructions (same engine
# queue, executed in order => semantics preserved) at BIR-serialization time.
_MAX_WAITS = 1


def _split_waits_json(bir: bytes) -> bytes:
    import orjson

    m = orjson.loads(bir)
    changed = False
    ctr = 0
    for f in m.get("functions", []):
        for b in f.get("blocks", []):
            out = []
            for i in b.get("instructions", []):
                si = i.get("sync_info")
                ow = (si or {}).get("on_wait") or []
                if len(ow) > _MAX_WAITS:
                    head = ow[: -_MAX_WAITS]
                    for j in range(0, len(head), _MAX_WAITS):
                        ctr += 1
                        out.append(
                            {
                                "debug": i.get("debug", 0),
                                "engine": i["engine"],
                                "ins": [],
                                "outs": [],
                                "name": f"{i['name']}-wsplit{ctr}",
                                "opcode": "NoOp",
                                "sync_info": {
                                    "on_wait": head[j : j + _MAX_WAITS],
                                    "on_update": [],
                                },
                            }
                        )
                    si["on_wait"] = ow[-_MAX_WAITS:]
                    changed = True
                out.append(i)
            b["instructions"] = out
    return orjson.dumps(m) if changed else bir


_orig_to_json_bytes = bass.Bass.to_json_bytes


def _patched_to_json_bytes(self):
    return _split_waits_json(_orig_to_json_bytes(self))


bass.Bass.to_json_bytes = _patched_to_json_bytes


def build_nc(BC, N, DZ, DA, E, H1, H2, has_b2, has_b3, has_g1, has_g2):
    """Build the per-core Bass program (SPMD, rank-free).

    BC batches per core. EL = E // NCORES local experts.
    """
    D = DZ + DA  # 192
    NT = N // 128  # 16 token tiles per batch
    EL = E // NCORES  # 2
    XA = D + 4  # x padded with 4 ones columns (col D = s_e trick)
    C1 = H1 // 128
    C2 = H2 // 128
    ROWS = EL * NCORES * BC  # 128 MLP pack rows = (el, c, b)
    assert DZ == 128 and E == 16 and BC == 8 and EL == 2 and ROWS == 128

    nc = bass.Bass(num_devices=NCORES)
    import os as _os
    for _ in range(int(_os.environ.get("K_SALT", "0"))):
        nc.sync.nop()

    xa = nc.dram_tensor("xa", [BC, 128, NT, XA], BF16, kind="ExternalInput")
    zTs = nc.dram_tensor("zTs", [BC, 128, NT, 128], BF16, kind="ExternalInput")
    aTs = nc.dram_tensor("aTs", [BC, DA, NT, 128], BF16, kind="ExternalInput")
    phi_d = nc.dram_tensor("phi_d", [D, E], BF16, kind="ExternalInput")
    w1h_d = nc.dram_tensor("w1h_d", [EL, 128, H1], BF16, kind="ExternalInput")
    w1l_d = nc.dram_tensor("w1l_d", [EL, D - 128 + 1, H1], BF16, kind="ExternalInput")
    w2_d = nc.dram_tensor("w2_d", [EL, C1, 128, H2], BF16, kind="ExternalInput")
    w3_d = nc.dram_tensor("w3_d", [EL, C2, 128, DZ], BF16, kind="ExternalInput")
    identb_d = nc.dram_tensor("identb_d", [128, 128], BF16, kind="ExternalInput")
    if has_b2:
        b2_d = nc.dram_tensor("b2_d", [1, EL * H2], BF16, kind="ExternalInput")
        ones_d = nc.dram_tensor("ones_d", [1, ROWS], BF16, kind="ExternalInput")
    if has_b3:
        b3_d = nc.dram_tensor("b3_d", [1, EL * DZ], BF16, kind="ExternalInput")
    if has_g1:
        g1_d = nc.dram_tensor("g1_d", [ROWS, H1], F32, kind="ExternalInput")
        be1_d = nc.dram_tensor("be1_d", [ROWS, H1], F32, kind="ExternalInput")
    if has_g2:
        g2_d = nc.dram_tensor("g2_d", [ROWS, H2], F32, kind="ExternalInput")
        be2_d = nc.dram_tensor("be2_d", [ROWS, H2], F32, kind="ExternalInput")
    out = nc.dram_tensor("out", [BC, 128, NT, DZ], F32, kind="ExternalOutput")

    LO = D - 128 + 1  # 65 rows: a-features 128..191 plus the s row

    with tile.TileContext(nc) as tc, ExitStack() as ctx:
        perm = ctx.enter_context(tc.tile_pool(name="perm", bufs=1))
        pa = ctx.enter_context(tc.tile_pool(name="pa", bufs=2))
        pec = ctx.enter_context(tc.tile_pool(name="pec", bufs=4))
        pm = ctx.enter_context(tc.tile_pool(name="pm", bufs=1))
        pst = ctx.enter_context(tc.tile_pool(name="pst", bufs=2))
        posb = ctx.enter_context(tc.tile_pool(name="posb", bufs=2))
        dram = ctx.enter_context(tc.tile_pool(name="dram", bufs=1, space="DRAM"))

        # ---------------- weight / constant prefetch (gpsimd queue) --------
        identb = perm.tile([128, 128], BF16)
        nc.gpsimd.dma_start(identb[:], identb_d[:, :])
        phi_hi = perm.tile([128, E], BF16)
        phi_lo = perm.tile([DA, E], BF16)
        nc.gpsimd.dma_start(phi_hi[:], phi_d[0:128, :])
        nc.gpsimd.dma_start(phi_lo[:], phi_d[128:D, :])
        w1h_sb = perm.tile([128, EL * H1], BF16)
        w1l_sb = perm.tile([LO, EL * H1], BF16)
        w2_sb = perm.tile([128, EL * C1 * H2], BF16)
        w3_sb = perm.tile([128, EL * C2 * DZ], BF16)
        nc.gpsimd.dma_start(
            w1h_sb[:].rearrange("p (el h) -> p el h", el=EL), w1h_d[:, :, :].rearrange("el p h -> p el h")
        )
        nc.gpsimd.dma_start(
            w1l_sb[:].rearrange("p (el h) -> p el h", el=EL), w1l_d[:, :, :].rearrange("el p h -> p el h")
        )
        nc.gpsimd.dma_start(
            w2_sb[:].rearrange("p (el c h) -> p el c h", el=EL, c=C1),
            w2_d[:, :, :, :].rearrange("el c p h -> p el c h"),
        )
        nc.gpsimd.dma_start(
            w3_sb[:].rearrange("p (el c h) -> p el c h", el=EL, c=C2),
            w3_d[:, :, :, :].rearrange("el c p h -> p el c h"),
        )
        if has_b2:
            b2_sb = perm.tile([1, EL * H2], BF16)
            nc.gpsimd.dma_start(b2_sb[:], b2_d[:, :])
            ones_sb = perm.tile([1, ROWS], BF16)
            nc.gpsimd.dma_start(ones_sb[:], ones_d[:, :])
        if has_b3:
            b3_sb = perm.tile([1, EL * DZ], BF16)
            nc.gpsimd.dma_start(b3_sb[:], b3_d[:, :])
        g1_sb = be1_sb = g2_sb = be2_sb = None
        if has_g1:
            g1_sb = perm.tile([ROWS, H1], F32)
            be1_sb = perm.tile([ROWS, H1], F32)
            nc.gpsimd.dma_start(g1_sb[:], g1_d[:, :])
            nc.gpsimd.dma_start(be1_sb[:], be1_d[:, :])
        if has_g2:
            g2_sb = perm.tile([ROWS, H2], F32)
            be2_sb = perm.tile([ROWS, H2], F32)
            nc.gpsimd.dma_start(g2_sb[:], g2_d[:, :])
            nc.gpsimd.dma_start(be2_sb[:], be2_d[:, :])

        # dummy collective: absorbs the first-collective warmup latency (the
        # first CC on this fabric costs ~15us extra) while phase A computes
        ccw_in = dram.tile([NCORES, 2], F32, name="ccw_in")
        ccw_out = dram.tile([NCORES, 2], F32, name="ccw_out")
        ccw_sb = perm.tile([NCORES, 2], F32)
        nc.gpsimd.memset(ccw_sb[:], 0.0)
        nc.gpsimd.dma_start(ccw_in[:], ccw_sb[:])
        for _ in range(2):
            nc.gpsimd.collective_compute(
                "AllToAll",
                mybir.AluOpType.bypass,
                replica_groups=[list(range(NCORES))],
                ins=[ccw_in[:].opt()],
                outs=[ccw_out[:].opt()],
            )

        # persistent phase-A outputs
        expCT = [perm.tile([E, N], BF16, name=f"expCT{b}") for b in range(BC)]
        rcomb = [perm.tile([128, NT], F32, name=f"rcomb{b}") for b in range(BC)]
        # xin packs split into batch halves so the first AllToAll can launch
        # while phase A still works on batches BC/2..BC-1
        HB = BC // 2
        xin_hi_h = [perm.tile([128, E * HB], BF16, name=f"xinh{h}") for h in range(2)]
        xin_lo_h = [perm.tile([LO, E * HB], BF16, name=f"xinl{h}") for h in range(2)]
        eps_col = perm.tile([128, 1], F32)
        nc.vector.memset(eps_col[:], LN_EPS)

        cc1_in = [
            dram.tile([NCORES, 128 + LO, EL * HB], BF16, name=f"cc1_in{h}")
            for h in range(2)
        ]
        cc1_out = [
            dram.tile([NCORES, 128 + LO, EL * HB], BF16, name=f"cc1_out{h}")
            for h in range(2)
        ]

        def emit_cc1(h):
            nc.gpsimd.dma_start(
                cc1_in[h][:, 0:128, :].rearrange("j d q -> d j q"),
                xin_hi_h[h][:].rearrange("p (j q) -> p j q", j=NCORES),
            )
            nc.gpsimd.dma_start(
                cc1_in[h][:, 128 : 128 + LO, :].rearrange("j d q -> d j q"),
                xin_lo_h[h][:].rearrange("p (j q) -> p j q", j=NCORES),
            )
            nc.gpsimd.collective_compute(
                "AllToAll",
                mybir.AluOpType.bypass,
                replica_groups=[list(range(NCORES))],
                ins=[cc1_in[h][:].opt()],
                outs=[cc1_out[h][:].opt()],
            )

        # ---------------- Phase A: routing + xin (data-parallel) -----------
        with tc.tile_pool(name="ps_wq", bufs=3, space="PSUM") as ps_wq, tc.tile_pool(
            name="ps_xin", bufs=2, space="PSUM"
        ) as ps_xin, tc.tile_pool(
            name="ps_xtr", bufs=1, space="PSUM"
        ) as ps_xtr:
            for b in range(BC):
                zT_sb = pa.tile([128, NT * 128], BF16, tag="zT")
                aT_sb = pa.tile([DA, NT * 128], BF16, tag="aT")
                nc.sync.dma_start(
                    zT_sb[:].rearrange("p (t c) -> p t c", c=128), zTs[b]
                )
                nc.sync.dma_start(
                    aT_sb[:].rearrange("p (t c) -> p t c", c=128), aTs[b]
                )
                x_sb = pa.tile([128, NT * XA], BF16, tag="x")
                xv = x_sb[:].rearrange("p (t c) -> p t c", c=XA)
                nc.sync.dma_start(xv[:, :, :], xa[b])
                zTv = zT_sb[:].rearrange("p (t c) -> p t c", c=128)
                aTv = aT_sb[:].rearrange("p (t c) -> p t c", c=128)

                xinps = ps_xin.tile([E, XA], F32, tag="xin")
                denom = pa.tile([128, NT], F32, tag="denom")
                for g in range(NT // 4):
                    # e-major routing: one [16, 512] chunk covers 4 token
                    # tiles; exp writes the combine lhsT (expCT) directly
                    sl512 = slice(512 * g, 512 * (g + 1))
                    wqe = ps_wq.tile([E, 512], F32, tag="wq")
                    nc.tensor.matmul(
                        wqe[:], phi_hi[:], zT_sb[:, sl512], start=True, stop=False
                    )
                    nc.tensor.matmul(
                        wqe[:], phi_lo[:], aT_sb[:, sl512], start=False, stop=True
                    )
                    nc.scalar.activation(expCT[b][:, sl512], wqe[:], AF.Exp)
                # one crossbar DMA transposes the whole [E, N] expCT into
                # token-major [128, (t, e)] tiles (no PE transposes/copies)
                ec_all = pec.tile([128, NT * E], BF16, tag="ec")
                nc.scalar.dma_start_transpose(
                    ec_all[:].rearrange("p (t e) -> p t e", t=NT), expCT[b][:]
                )
                ecv = ec_all[:].rearrange("p (t e) -> p t e", t=NT)
                for t in range(NT):
                    # xin accumulation (contract tokens)
                    nc.tensor.matmul(
                        xinps[:],
                        ecv[:, t, :],
                        xv[:, t, :],
                        start=(t == 0),
                        stop=(t == NT - 1),
                    )
                # combine denominators: one reduce over all 16 tiles
                nc.vector.tensor_reduce(
                    denom[:],
                    ecv[:, :, :],
                    mybir.AxisListType.X,
                    mybir.AluOpType.add,
                )
                # reciprocal of combine denominators (all 16 tiles at once)
                nc.vector.reciprocal(rcomb[b][:], denom[:])
                # xin -> bf16 -> transposed into the [d, (e b-half)] packs
                xin_sb = pec.tile([E, XA], BF16, tag="xin_sb")
                nc.scalar.copy(xin_sb[:], xinps[:])
                h, bb = b // HB, b % HB
                xhv = xin_hi_h[h][:].rearrange("p (e b) -> p e b", b=HB)
                xlv = xin_lo_h[h][:].rearrange("p (e b) -> p e b", b=HB)
                pth = ps_xtr.tile([128, E], BF16, tag="trh")
                nc.tensor.transpose(pth[:], xin_sb[:, 0:128], identb[0:E, 0:E])
                nc.vector.tensor_copy(xhv[:, :, bb], pth[:])
                ptl = ps_xtr.tile([LO + 1, E], BF16, tag="trl")
                nc.tensor.transpose(
                    ptl[:], xin_sb[:, 128 : 128 + LO + 1], identb[0:E, 0:E]
                )
                nc.scalar.copy(xlv[:, :, bb], ptl[0:LO, :])
                if b == HB - 1:
                    emit_cc1(0)

        # xinp cols: (h, el, c, bb) -- MLP rows follow the same order, so the
        # L1 matmuls for half 0 can run while phase A / cc1b still execute
        xinp_hi_h = [
            perm.tile([128, ROWS // 2], BF16, name=f"xph{h}") for h in range(2)
        ]
        xinp_lo_h2 = [
            perm.tile([LO, ROWS // 2], BF16, name=f"xpl{h}") for h in range(2)
        ]

        def emit_cc1_load(h):
            for el in range(EL):
                csl = slice(32 * el, 32 * (el + 1))
                qsl = slice(HB * el, HB * (el + 1))
                eng = nc.gpsimd if el == 0 else nc.sync
                eng2 = nc.gpsimd if el == 0 else nc.scalar
                eng.dma_start(
                    xinp_hi_h[h][:, csl].rearrange("p (c b) -> p c b", c=NCORES),
                    cc1_out[h][:, 0:128, qsl].rearrange("c d b -> d c b"),
                )
                eng2.dma_start(
                    xinp_lo_h2[h][:, csl].rearrange("p (c b) -> p c b", c=NCORES),
                    cc1_out[h][:, 128 : 128 + LO, qsl].rearrange("c d b -> d c b"),
                )
            # r for this half: 1/s from the s row of xinp_lo
            nc.vector.tensor_copy(
                r_row[0:1, 64 * h : 64 * (h + 1)], xinp_lo_h2[h][LO - 1 : LO, :]
            )
            nc.vector.reciprocal(
                r_row[0:1, 64 * h : 64 * (h + 1)], r_row[0:1, 64 * h : 64 * (h + 1)]
            )
            nc.gpsimd.dma_start(
                r_col[64 * h : 64 * (h + 1), :], r_row[0:1, 64 * h : 64 * (h + 1)]
            )

        # r = 1/s per MLP row (h, el, c, bb); s sits in xinp_lo row LO-1
        r_row = perm.tile([1, ROWS], F32)
        r_col = perm.tile([128, 1], F32)

        # ---------------- MLP (expert-parallel, rows = (el, c, b)) ---------
        def ln_mish(hs, H, gr, ber):
            """LayerNorm + mish of SBUF [128, H] f32 -> bf16.

            Uses only Exp/Ln/Square activations (one act table, no reload):
              rstd = exp(-0.5 * ln(var + eps))
              mish(x) = x * tanh(ln(u)), u = 1 + e^x
                      = x * (1 - 2 * exp(-ln(u^2 + 1)))
            """
            stats = pm.tile([128, 6], F32, tag="stats")
            nc.vector.bn_stats(stats[:], hs)
            mv = pm.tile([128, 2], F32, tag="mv")
            nc.vector.bn_aggr(mv[:], stats[:])
            lnv = pm.tile([128, 1], F32, tag="lnv")
            nc.scalar.activation(lnv[:], mv[:, 1:2], AF.Ln, bias=eps_col[:])
            rstd = pm.tile([128, 1], F32, tag="rstd")
            nc.scalar.activation(rstd[:], lnv[:], AF.Exp, scale=-0.5)
            xn = pm.tile([128, H], F32, tag="xn")
            nc.vector.tensor_scalar(
                xn[:], hs, mv[:, 0:1], rstd[:],
                mybir.AluOpType.subtract, mybir.AluOpType.mult,
            )
            if gr is not None:
                xg = pm.tile([128, H], F32, tag="xg")
                nc.vector.tensor_mul(xg[:], xn[:], gr)
                xn = pm.tile([128, H], F32, tag="xb")
                nc.vector.tensor_add(xn[:], xg[:], ber)
            ex = pm.tile([128, H], F32, tag="ex")
            nc.scalar.activation(ex[:], xn[:], AF.Exp)
            sq2 = pm.tile([128, H], F32, tag="sq2")
            nc.scalar.activation(sq2[:], ex[:], AF.Square, bias=1.0)
            ln2 = pm.tile([128, H], F32, tag="ln2")
            nc.scalar.activation(ln2[:], sq2[:], AF.Ln, bias=1.0)
            wv = pm.tile([128, H], F32, tag="wv")
            nc.scalar.activation(wv[:], ln2[:], AF.Exp, scale=-1.0)
            m = pm.tile([128, H], F32, tag="m")
            nc.vector.tensor_scalar(
                m[:], wv[:], -2.0, 1.0, mybir.AluOpType.mult, mybir.AluOpType.add
            )
            hm = pm.tile([128, H], BF16, tag="hm")
            nc.vector.tensor_mul(hm[:], xn[:], m[:])
            return hm

        w1h_v = w1h_sb[:].rearrange("p (el h) -> p el h", el=EL)
        w1l_v = w1l_sb[:].rearrange("p (el h) -> p el h", el=EL)
        w2_v = w2_sb[:].rearrange("p (el c h) -> p el c h", el=EL, c=C1)
        w3_v = w3_sb[:].rearrange("p (el c h) -> p el c h", el=EL, c=C2)

        with tc.tile_pool(name="ps_mlp", bufs=2, space="PSUM") as ps_mlp, tc.tile_pool(
            name="ps_eo", bufs=1, space="PSUM"
        ) as ps_eo:
            h1ps = ps_mlp.tile([128, H1], F32, tag="h12")

            def emit_l1(h):
                for el in range(EL):
                    osl = slice(64 * h + 32 * el, 64 * h + 32 * (el + 1))
                    csl = slice(32 * el, 32 * (el + 1))
                    tp = (0, 64 * h + 32 * el)
                    nc.tensor.matmul(
                        h1ps[osl, :],
                        xinp_hi_h[h][:, csl],
                        w1h_v[:, el, :],
                        start=True,
                        stop=False,
                        tile_position=tp,
                    )
                    nc.tensor.matmul(
                        h1ps[osl, :],
                        xinp_lo_h2[h][:, csl],
                        w1l_v[:, el, :],
                        start=False,
                        stop=True,
                        tile_position=tp,
                    )

            # half 0: loads + L1 while cc1b is still in flight
            emit_cc1_load(0)
            emit_l1(0)
            emit_cc1(1)
            emit_cc1_load(1)
            emit_l1(1)
            h1s = pm.tile([128, H1], F32, tag="h1s")
            nc.vector.tensor_scalar_mul(h1s[:], h1ps[:], r_col[:])
            h1m = ln_mish(h1s[:], H1, g1_sb[:] if has_g1 else None, be1_sb[:] if has_g1 else None)
            h1T = pm.tile([128, C1 * 128], BF16, tag="h1T")
            nc.sync.dma_start_transpose(
                h1T[:].rearrange("p (c m) -> p c m", c=C1), h1m[:]
            )

            h2ps = ps_mlp.tile([128, H2], F32, tag="h12")
            for h in range(2):
                for el in range(EL):
                    osl = slice(64 * h + 32 * el, 64 * h + 32 * (el + 1))
                    tp = (0, 64 * h + 32 * el)
                    for c in range(C1):
                        nc.tensor.matmul(
                            h2ps[osl, :],
                            h1T[:, c * 128 + 64 * h + 32 * el : c * 128 + 64 * h + 32 * (el + 1)],
                            w2_v[:, el, c, :],
                            start=(c == 0),
                            stop=(c == C1 - 1 and not has_b2),
                            tile_position=tp,
                        )
                    if has_b2:
                        nc.tensor.matmul(
                            h2ps[osl, :],
                            ones_sb[0:1, 0:32],
                            b2_sb[0:1, el * H2 : (el + 1) * H2],
                            start=False,
                            stop=True,
                            tile_position=tp,
                        )
            h2s = pm.tile([128, H2], F32, tag="h2s")
            nc.vector.tensor_copy(h2s[:], h2ps[:])
            h2m = ln_mish(h2s[:], H2, g2_sb[:] if has_g2 else None, be2_sb[:] if has_g2 else None)
            h2T = pm.tile([128, C2 * 128], BF16, tag="h2T")
            nc.sync.dma_start_transpose(
                h2T[:].rearrange("p (c m) -> p c m", c=C2), h2m[:]
            )

            eops = ps_eo.tile([128, DZ], F32, tag="eo")
            for h in range(2):
                for el in range(EL):
                    osl = slice(64 * h + 32 * el, 64 * h + 32 * (el + 1))
                    tp = (0, 64 * h + 32 * el)
                    for c in range(C2):
                        nc.tensor.matmul(
                            eops[osl, :],
                            h2T[:, c * 128 + 64 * h + 32 * el : c * 128 + 64 * h + 32 * (el + 1)],
                            w3_v[:, el, c, :],
                            start=(c == 0),
                            stop=(c == C2 - 1 and not has_b3),
                            tile_position=tp,
                        )
                    if has_b3:
                        nc.tensor.matmul(
                            eops[osl, :],
                            ones_sb[0:1, 0:32],
                            b3_sb[0:1, el * DZ : (el + 1) * DZ],
                            start=False,
                            stop=True,
                            tile_position=tp,
                        )
            eo_sb = pm.tile([128, DZ], BF16, tag="eo_sb")
            nc.vector.tensor_copy(eo_sb[:], eops[:])

        # ---------------- AllToAll 2: expert outputs back ------------------
        cc2_in = dram.tile([NCORES, EL, BC, DZ], BF16)
        cc2_out = dram.tile([NCORES, EL, BC, DZ], BF16)
        for h in range(2):
            for el in range(EL):
                eng = (nc.gpsimd, nc.scalar, nc.sync, nc.gpsimd)[2 * h + el]
                eng.dma_start(
                    cc2_in[:, el, HB * h : HB * (h + 1), :],
                    eo_sb[64 * h + 32 * el : 64 * h + 32 * (el + 1), :],
                )
        nc.gpsimd.collective_compute(
            "AllToAll",
            mybir.AluOpType.bypass,
            replica_groups=[list(range(NCORES))],
            ins=[cc2_in[:].opt()],
            outs=[cc2_out[:].opt()],
        )
        eo_b = [perm.tile([E, DZ], BF16, name=f"eo{b}") for b in range(BC)]
        for b in range(BC):
            eng = (nc.gpsimd, nc.scalar, nc.sync)[b % 3]
            eng.dma_start(eo_b[b][:], cc2_out[:, :, b, :])

        # ---------------- Combine (data-parallel) --------------------------
        with tc.tile_pool(name="ps_cmb", bufs=4, space="PSUM") as ps_cmb:
            for b in range(BC):
                osb = posb.tile([128, NT * DZ], F32, tag="osb")
                ov = osb[:].rearrange("p (t d) -> p t d", d=DZ)
                for t in range(NT):
                    cps = ps_cmb.tile([128, DZ], F32, tag="c")
                    nc.tensor.matmul(
                        cps[:],
                        expCT[b][:, 128 * t : 128 * (t + 1)],
                        eo_b[b][:],
                        start=True,
                        stop=True,
                    )
                    eng = (nc.vector, nc.scalar)[t % 2]
                    if eng is nc.scalar:
                        eng.mul(ov[:, t, :], cps[:], rcomb[b][:, t : t + 1])
                    else:
                        eng.tensor_scalar_mul(
                            ov[:, t, :], cps[:], rcomb[b][:, t : t + 1]
                        )
                eng = nc.gpsimd if b % 2 == 0 else nc.sync
                eng.dma_start(out[b], ov[:, :, :])
    return nc


# ---------------------------------------------------------------------------
# Host wrapper
# ---------------------------------------------------------------------------

_CACHE = {}


def _get_nc(key, *args):
    if key not in _CACHE:
        _CACHE[key] = build_nc(*args)
    return _CACHE[key]


def _prepare(z, a, phi, W1, b1, g1, be1, W2, b2, g2, be2, W3, b3):
    """Build (cached) the Bass program and per-core input maps."""
    import ml_dtypes

    BF = ml_dtypes.bfloat16

    z = np.asarray(z, np.float32)
    a = np.asarray(a, np.float32)
    phi = np.asarray(phi, np.float32)
    W1 = np.asarray(W1, np.float32)
    b1 = np.asarray(b1, np.float32)
    g1 = np.asarray(g1, np.float32)
    be1 = np.asarray(be1, np.float32)
    W2 = np.asarray(W2, np.float32)
    b2 = np.asarray(b2, np.float32)
    g2 = np.asarray(g2, np.float32)
    be2 = np.asarray(be2, np.float32)
    W3 = np.asarray(W3, np.float32)
    b3 = np.asarray(b3, np.float32)

    B, N, DZ = z.shape
    DA = a.shape[2]
    D = DZ + DA
    E = W1.shape[0]
    H1 = W1.shape[2]
    H2 = W2.shape[2]
    BC = B // NCORES
    EL = E // NCORES
    NT = N // 128
    XA = D + 4

    has_b2 = bool(np.any(b2))
    has_b3 = bool(np.any(b3))
    has_g1 = not (np.all(g1 == 1.0) and np.all(be1 == 0.0))
    has_g2 = not (np.all(g2 == 1.0) and np.all(be2 == 0.0))

    key = (BC, N, DZ, DA, E, H1, H2, has_b2, has_b3, has_g1, has_g2)
    nc = _get_nc(key, *key)

    # x native, padded with ones: [B, N, XA] -> [B, 128, NT, XA]
    xf = np.empty((B, N, XA), np.float32)
    xf[:, :, 0:DZ] = z
    xf[:, :, DZ:D] = a
    xf[:, :, D:XA] = 1.0
    xa_all = np.ascontiguousarray(
        xf.reshape(B, 128, NT, XA)
    ).astype(BF)  # token n = p*NT + t
    # x transposed, tile-shuffled: [B, D, NT, 128] with col (t, p)
    xT = np.concatenate([z, a], axis=-1).transpose(0, 2, 1)  # [B, D, N]
    xTs = xT.reshape(B, D, 128, NT).transpose(0, 1, 3, 2)  # [B, D, NT, 128]
    zTs_all = np.ascontiguousarray(xTs[:, 0:128]).astype(BF)
    aTs_all = np.ascontiguousarray(xTs[:, 128:D]).astype(BF)

    phi2 = np.ascontiguousarray(phi.reshape(D, -1)).astype(BF)
    w1aug = np.concatenate([W1, b1[:, None, :]], axis=1)  # [E, D+1, H1]
    identb = np.eye(128, dtype=np.float32).astype(BF)

    in_maps = []
    for i in range(NCORES):
        es = slice(i * EL, (i + 1) * EL)
        m = {
            "xa": xa_all[i * BC : (i + 1) * BC],
            "zTs": zTs_all[i * BC : (i + 1) * BC],
            "aTs": aTs_all[i * BC : (i + 1) * BC],
            "phi_d": phi2,
            "w1h_d": np.ascontiguousarray(w1aug[es, 0:128]).astype(BF),
            "w1l_d": np.ascontiguousarray(w1aug[es, 128 : D + 1]).astype(BF),
            "w2_d": np.ascontiguousarray(
                W2[es].reshape(EL, H1 // 128, 128, H2)
            ).astype(BF),
            "w3_d": np.ascontiguousarray(
                W3[es].reshape(EL, H2 // 128, 128, DZ)
            ).astype(BF),
            "identb_d": identb,
        }
        if has_b2:
            m["b2_d"] = np.ascontiguousarray(b2[es].reshape(1, -1)).astype(BF)
            m["ones_d"] = np.ones((1, 128), np.float32).astype(BF)
        if has_b3:
            m["b3_d"] = np.ascontiguousarray(b3[es].reshape(1, -1)).astype(BF)
        if has_g1:
            m["g1_d"] = np.ascontiguousarray(np.tile(np.repeat(g1[es], 32, 0), (2, 1)))
            m["be1_d"] = np.ascontiguousarray(np.tile(np.repeat(be1[es], 32, 0), (2, 1)))
        if has_g2:
            m["g2_d"] = np.ascontiguousarray(np.tile(np.repeat(g2[es], 32, 0), (2, 1)))
            m["be2_d"] = np.ascontiguousarray(np.tile(np.repeat(be2[es], 32, 0), (2, 1)))
        in_maps.append(m)
    return nc, in_maps


def kernel(**inputs):
    nc, in_maps = _prepare(**inputs)

    from concourse.bass_utils import run_bass_kernel_spmd

    res = run_bass_kernel_spmd(nc, in_maps, list(range(NCORES)))
    B = len(in_maps) * in_maps[0]["xa"].shape[0]
    outs = []
    for r in res.results:
        o = r["out"]  # [BC, 128, NT, DZ], token n = p*NT + t
        BC, P, NT_, DZ_ = o.shape
        outs.append(o.reshape(BC, P * NT_, DZ_))
    return np.concatenate(outs, axis=0)


# revision 28
# speedup vs baseline: 1.2829x; 1.2766x over previous
"""Trainium2 Bass kernel for nn_CenMoEDynamicsModel (MoE routing), v2.

Contract: kernel(**inputs) takes FULL unsharded numpy inputs and returns the
FULL [64, 2048, 128] f32 output. Data-parallel over B across 8 NeuronCores for
routing + combine; expert-parallel (2 experts/core) for the MLP with two tiny
AllToAll exchanges (xin out, expert_outputs back).

Math (per batch b):
  x = [z|a]                       [N, D]     D = 192
  w = x @ phi                     [N, E]     E = 16
  dispatch = softmax_n(w); xin = dispatch^T @ x          [E, D]
  h = mish(LN(xin@W1+b1)); h = mish(LN(h@W2+b2)); EO = h@W3+b3   [E, DZ]
  combine = softmax_e(w); out = combine @ EO             [N, DZ]

Key design points (vs v1 baseline):
  - everything bf16 on the PE (1 cyc/row always; f32r pays 4 cyc/row for
    moving dims < 256) and half the HBM/DMA bytes.
  - token-major routing: out[128tok, E] streams only E=16 rows per matmul
    (vs 512 for e-major), and exp() writes the token-major ec tiles the xin
    matmul needs directly. e-major expCT (combine lhsT) comes from 16 cheap
    PE transposes per batch.
  - combine softmax denominators reduced from ec tiles in phase A (DVE),
    so combine is matmul + one scale/evacuation op per tile, no ones column.
  - expert weights sharded: each core loads 2 experts (1.7 MB bf16 vs 27 MB
    f32 for all 16). xin packs are exchanged with AllToAll ([8,193,16] bf16),
    expert outputs return with a second AllToAll ([8,16,128] bf16).
  - MLP activations packed [2 experts x 64 batches, H] = full 128 partitions;
    PE writes both experts into one PSUM tile (offsets 0/64). Inter-layer
    transposes use the DMA crossbar (dma_start_transpose), not the PE.
  - single top-level SBUF pool scope => weight prefetch DMAs (gpsimd queue)
    overlap phase A input streaming (sync queue); no inter-phase SBUF
    anti-dependency barriers.

ones-column tricks kept from v1: 4 ones columns appended to x give the
dispatch denominator s_e as xin row D; W1 is bias-augmented so scaling L1 rows
by r = 1/s normalizes dispatch and bias exactly.
"""

import sys

import numpy as np

sys.path.insert(0, "/opt/trn_rl_repo")

from contextlib import ExitStack

import concourse.bass as bass
import concourse.tile as tile
from concourse import mybir

F32 = mybir.dt.float32
BF16 = mybir.dt.bfloat16
AF = mybir.ActivationFunctionType

LN_EPS = 1e-5
NCORES = 8


def _split_drain_and_barrier(self, tick_clock, wait_clock):
    """Replacement for TileContext._drain_and_barrier.

    The stock version attaches every outstanding semaphore wait to ONE tail
    Drain instruction; this walrus build's codegen rejects Drains with more
    than a couple of sync waits ("Too many sync wait commands"). Emit one
    single-wait Drain per logical proc instead (the SP queue executes them in
    order, so the final bare drain still happens after everything finished).
    """
    from concourse.vector_clock import ScopedClock, VectorClock

    nc = self.nc
    gc = tick_clock.global_clock
    n = len(gc)
    for i in range(n):
        t = gc[i]
        if t <= 0:
            continue
        v = VectorClock([0] * n)
        v.require_at_least(i, t)
        d = nc.sync.drain()
        wait_clock.add_sem_waits(d.ins, ScopedClock({None: v}))
    nc.sync.drain()
    nc.all_engine_barrier()
    assert self.sems is not None
    popped = nc._tile_sem_poison_stack.pop()
    assert popped is self._sem_poison
    nc.clear_and_free_semaphores(list(self.sems.allocated().values()))
    nc.all_engine_barrier()


tile.TileContext._drain_and_barrier = _split_drain_and_barrier

# This walrus build rejects instructions carrying more than a couple of sync
# waits ("Too many sync wait commands" in CoreV3 codegen), while Tile freely
# attaches 3+. Split excess waits onto NoOp carrier instructions (same engine
# queue, executed in order => semantics preserved) at BIR-serialization time.
_MAX_WAITS = 1


def _split_waits_json(bir: bytes) -> bytes:
    import orjson

    m = orjson.loads(bir)
    changed = False
    ctr = 0
    for f in m.get("functions", []):
        for b in f.get("blocks", []):
            out = []
            for i in b.get("instructions", []):
                si = i.get("sync_info")
                ow = (si or {}).get("on_wait") or []
                if len(ow) > _MAX_WAITS:
                    head = ow[: -_MAX_WAITS]
                    for j in range(0, len(head), _MAX_WAITS):
                        ctr += 1
                        out.append(
                            {
                                "debug": i.get("debug", 0),
                                "engine": i["engine"],
                                "ins": [],
                                "outs": [],
                                "name": f"{i['name']}-wsplit{ctr}",
                                "opcode": "NoOp",
                                "sync_info": {
                                    "on_wait": head[j : j + _MAX_WAITS],
                                    "on_update": [],
                                },
                            }
                        )
                    si["on_wait"] = ow[-_MAX_WAITS:]
                    changed = True
                out.append(i)
            b["instructions"] = out
    return orjson.dumps(m) if changed else bir


_orig_to_json_bytes = bass.Bass.to_json_bytes


def _patched_to_json_bytes(self):
    return _split_waits_json(_orig_to_json_bytes(self))


bass.Bass.to_json_bytes = _patched_to_json_bytes


def build_nc(BC, N, DZ, DA, E, H1, H2, has_b2, has_b3, has_g1, has_g2):
    """Build the per-core Bass program (SPMD, rank-free).

    BC batches per core. EL = E // NCORES local experts.
    """
    D = DZ + DA  # 192
    NT = N // 128  # 16 token tiles per batch
    EL = E // NCORES  # 2
    XA = D + 4  # x padded with 4 ones columns (col D = s_e trick)
    C1 = H1 // 128
    C2 = H2 // 128
    ROWS = EL * NCORES * BC  # 128 MLP pack rows = (el, c, b)
    assert DZ == 128 and E == 16 and BC == 8 and EL == 2 and ROWS == 128

    nc = bass.Bass(num_devices=NCORES)
    import os as _os
    for _ in range(int(_os.environ.get("K_SALT", "0"))):
        nc.sync.nop()

    xa = nc.dram_tensor("xa", [BC, 128, NT, XA], BF16, kind="ExternalInput")
    zTs = nc.dram_tensor("zTs", [BC, 128, NT, 128], BF16, kind="ExternalInput")
    aTs = nc.dram_tensor("aTs", [BC, DA, NT, 128], BF16, kind="ExternalInput")
    phi_d = nc.dram_tensor("phi_d", [D, E], BF16, kind="ExternalInput")
    w1h_d = nc.dram_tensor("w1h_d", [EL, 128, H1], BF16, kind="ExternalInput")
    w1l_d = nc.dram_tensor("w1l_d", [EL, D - 128 + 1, H1], BF16, kind="ExternalInput")
    w2_d = nc.dram_tensor("w2_d", [EL, C1, 128, H2], BF16, kind="ExternalInput")
    w3_d = nc.dram_tensor("w3_d", [EL, C2, 128, DZ], BF16, kind="ExternalInput")
    identb_d = nc.dram_tensor("identb_d", [128, 128], BF16, kind="ExternalInput")
    if has_b2:
        b2_d = nc.dram_tensor("b2_d", [1, EL * H2], BF16, kind="ExternalInput")
        ones_d = nc.dram_tensor("ones_d", [1, ROWS], BF16, kind="ExternalInput")
    if has_b3:
        b3_d = nc.dram_tensor("b3_d", [1, EL * DZ], BF16, kind="ExternalInput")
    if has_g1:
        g1_d = nc.dram_tensor("g1_d", [ROWS, H1], F32, kind="ExternalInput")
        be1_d = nc.dram_tensor("be1_d", [ROWS, H1], F32, kind="ExternalInput")
    if has_g2:
        g2_d = nc.dram_tensor("g2_d", [ROWS, H2], F32, kind="ExternalInput")
        be2_d = nc.dram_tensor("be2_d", [ROWS, H2], F32, kind="ExternalInput")
    out = nc.dram_tensor("out", [BC, 128, NT, DZ], F32, kind="ExternalOutput")

    LO = D - 128 + 1  # 65 rows: a-features 128..191 plus the s row

    with tile.TileContext(nc) as tc, ExitStack() as ctx:
        perm = ctx.enter_context(tc.tile_pool(name="perm", bufs=1))
        pa = ctx.enter_context(tc.tile_pool(name="pa", bufs=2))
        pec = ctx.enter_context(tc.tile_pool(name="pec", bufs=4))
        pm = ctx.enter_context(tc.tile_pool(name="pm", bufs=1))
        pst = ctx.enter_context(tc.tile_pool(name="pst", bufs=2))
        posb = ctx.enter_context(tc.tile_pool(name="posb", bufs=2))
        dram = ctx.enter_context(tc.tile_pool(name="dram", bufs=1, space="DRAM"))

        # ---------------- weight / constant prefetch (gpsimd queue) --------
        identb = perm.tile([128, 128], BF16)
        nc.gpsimd.dma_start(identb[:], identb_d[:, :])
        phi_hi = perm.tile([128, E], BF16)
        phi_lo = perm.tile([DA, E], BF16)
        nc.gpsimd.dma_start(phi_hi[:], phi_d[0:128, :])
        nc.gpsimd.dma_start(phi_lo[:], phi_d[128:D, :])
        w1h_sb = perm.tile([128, EL * H1], BF16)
        w1l_sb = perm.tile([LO, EL * H1], BF16)
        w2_sb = perm.tile([128, EL * C1 * H2], BF16)
        w3_sb = perm.tile([128, EL * C2 * DZ], BF16)
        nc.gpsimd.dma_start(
            w1h_sb[:].rearrange("p (el h) -> p el h", el=EL), w1h_d[:, :, :].rearrange("el p h -> p el h")
        )
        nc.gpsimd.dma_start(
            w1l_sb[:].rearrange("p (el h) -> p el h", el=EL), w1l_d[:, :, :].rearrange("el p h -> p el h")
        )
        nc.gpsimd.dma_start(
            w2_sb[:].rearrange("p (el c h) -> p el c h", el=EL, c=C1),
            w2_d[:, :, :, :].rearrange("el c p h -> p el c h"),
        )
        nc.gpsimd.dma_start(
            w3_sb[:].rearrange("p (el c h) -> p el c h", el=EL, c=C2),
            w3_d[:, :, :, :].rearrange("el c p h -> p el c h"),
        )
        if has_b2:
            b2_sb = perm.tile([1, EL * H2], BF16)
            nc.gpsimd.dma_start(b2_sb[:], b2_d[:, :])
            ones_sb = perm.tile([1, ROWS], BF16)
            nc.gpsimd.dma_start(ones_sb[:], ones_d[:, :])
        if has_b3:
            b3_sb = perm.tile([1, EL * DZ], BF16)
            nc.gpsimd.dma_start(b3_sb[:], b3_d[:, :])
        g1_sb = be1_sb = g2_sb = be2_sb = None
        if has_g1:
            g1_sb = perm.tile([ROWS, H1], F32)
            be1_sb = perm.tile([ROWS, H1], F32)
            nc.gpsimd.dma_start(g1_sb[:], g1_d[:, :])
            nc.gpsimd.dma_start(be1_sb[:], be1_d[:, :])
        if has_g2:
            g2_sb = perm.tile([ROWS, H2], F32)
            be2_sb = perm.tile([ROWS, H2], F32)
            nc.gpsimd.dma_start(g2_sb[:], g2_d[:, :])
            nc.gpsimd.dma_start(be2_sb[:], be2_d[:, :])

        # dummy collective: absorbs the first-collective warmup latency (the
        # first CC on this fabric costs ~15us extra) while phase A computes
        ccw_in = dram.tile([NCORES, 2], F32, name="ccw_in")
        ccw_out = dram.tile([NCORES, 2], F32, name="ccw_out")
        ccw_sb = perm.tile([NCORES, 2], F32)
        nc.gpsimd.memset(ccw_sb[:], 0.0)
        nc.gpsimd.dma_start(ccw_in[:], ccw_sb[:])
        for _ in range(2):
            nc.gpsimd.collective_compute(
                "AllToAll",
                mybir.AluOpType.bypass,
                replica_groups=[list(range(NCORES))],
                ins=[ccw_in[:].opt()],
                outs=[ccw_out[:].opt()],
            )

        # persistent phase-A outputs
        expCT = [perm.tile([E, N], BF16, name=f"expCT{b}") for b in range(BC)]
        rcomb = [perm.tile([128, NT], F32, name=f"rcomb{b}") for b in range(BC)]
        # xin packs split into batch halves so the first AllToAll can launch
        # while phase A still works on batches BC/2..BC-1
        HB = BC // 2
        xin_hi_h = [perm.tile([128, E * HB], BF16, name=f"xinh{h}") for h in range(2)]
        xin_lo_h = [perm.tile([LO, E * HB], BF16, name=f"xinl{h}") for h in range(2)]
        eps_col = perm.tile([128, 1], F32)
        nc.vector.memset(eps_col[:], LN_EPS)

        cc1_in = [
            dram.tile([NCORES, 128 + LO, EL * HB], BF16, name=f"cc1_in{h}")
            for h in range(2)
        ]
        cc1_out = [
            dram.tile([NCORES, 128 + LO, EL * HB], BF16, name=f"cc1_out{h}")
            for h in range(2)
        ]

        def emit_cc1(h):
            nc.gpsimd.dma_start(
                cc1_in[h][:, 0:128, :].rearrange("j d q -> d j q"),
                xin_hi_h[h][:].rearrange("p (j q) -> p j q", j=NCORES),
            )
            nc.gpsimd.dma_start(
                cc1_in[h][:, 128 : 128 + LO, :].rearrange("j d q -> d j q"),
                xin_lo_h[h][:].rearrange("p (j q) -> p j q", j=NCORES),
            )
            nc.gpsimd.collective_compute(
                "AllToAll",
                mybir.AluOpType.bypass,
                replica_groups=[list(range(NCORES))],
                ins=[cc1_in[h][:].opt()],
                outs=[cc1_out[h][:].opt()],
            )

        # ---------------- Phase A: routing + xin (data-parallel) -----------
        with tc.tile_pool(name="ps_wq", bufs=2, space="PSUM") as ps_wq, tc.tile_pool(
            name="ps_ect", bufs=2, space="PSUM"
        ) as ps_ect, tc.tile_pool(name="ps_xin", bufs=2, space="PSUM") as ps_xin, tc.tile_pool(
            name="ps_xtr", bufs=1, space="PSUM"
        ) as ps_xtr:
            for b in range(BC):
                zT_sb = pa.tile([128, NT * 128], BF16, tag="zT")
                aT_sb = pa.tile([DA, NT * 128], BF16, tag="aT")
                nc.sync.dma_start(
                    zT_sb[:].rearrange("p (t c) -> p t c", c=128), zTs[b]
                )
                nc.sync.dma_start(
                    aT_sb[:].rearrange("p (t c) -> p t c", c=128), aTs[b]
                )
                x_sb = pa.tile([128, NT * XA], BF16, tag="x")
                xv = x_sb[:].rearrange("p (t c) -> p t c", c=XA)
                nc.sync.dma_start(xv[:, :, :], xa[b])
                zTv = zT_sb[:].rearrange("p (t c) -> p t c", c=128)
                aTv = aT_sb[:].rearrange("p (t c) -> p t c", c=128)

                xinps = ps_xin.tile([E, XA], F32, tag="xin")
                denom = pa.tile([128, NT], F32, tag="denom")
                for g in range(NT // 4):
                    # e-major routing: one [16, 512] chunk covers 4 token
                    # tiles; exp writes the combine lhsT (expCT) directly
                    sl512 = slice(512 * g, 512 * (g + 1))
                    wqe = ps_wq.tile([E, 512], F32, tag="wq")
                    nc.tensor.matmul(
                        wqe[:], phi_hi[:], zT_sb[:, sl512], start=True, stop=False
                    )
                    nc.tensor.matmul(
                        wqe[:], phi_lo[:], aT_sb[:, sl512], start=False, stop=True
                    )
                    nc.scalar.activation(expCT[b][:, sl512], wqe[:], AF.Exp)
                    # token-major ec tiles for the xin contraction
                    ec4 = pec.tile([128, 64], BF16, tag="ec")
                    for j in range(4):
                        t = g * 4 + j
                        sl = slice(16 * j, 16 * (j + 1))
                        ecp = ps_ect.tile([128, E], BF16, tag="ect")
                        nc.tensor.transpose(
                            ecp[:],
                            expCT[b][:, 128 * t : 128 * (t + 1)],
                            identb[0:E, 0:E],
                        )
                        eng = nc.vector if j % 2 == 0 else nc.scalar
                        cpy = eng.copy if eng is nc.scalar else eng.tensor_copy
                        cpy(ec4[:, sl], ecp[:])
                        # xin accumulation (contract tokens)
                        nc.tensor.matmul(
                            xinps[:],
                            ec4[:, sl],
                            xv[:, t, :],
                            start=(t == 0),
                            stop=(t == NT - 1),
                        )
                    # combine denominators for these 4 tiles
                    nc.vector.tensor_reduce(
                        denom[:, 4 * g : 4 * (g + 1)],
                        ec4[:].rearrange("p (u e) -> p u e", u=4),
                        mybir.AxisListType.X,
                        mybir.AluOpType.add,
                    )
                # reciprocal of combine denominators (all 16 tiles at once)
                nc.vector.reciprocal(rcomb[b][:], denom[:])
                # xin -> bf16 -> transposed into the [d, (e b-half)] packs
                xin_sb = pec.tile([E, XA], BF16, tag="xin_sb")
                nc.scalar.copy(xin_sb[:], xinps[:])
                h, bb = b // HB, b % HB
                xhv = xin_hi_h[h][:].rearrange("p (e b) -> p e b", b=HB)
                xlv = xin_lo_h[h][:].rearrange("p (e b) -> p e b", b=HB)
                pth = ps_xtr.tile([128, E], BF16, tag="trh")
                nc.tensor.transpose(pth[:], xin_sb[:, 0:128], identb[0:E, 0:E])
                nc.vector.tensor_copy(xhv[:, :, bb], pth[:])
                ptl = ps_xtr.tile([LO + 1, E], BF16, tag="trl")
                nc.tensor.transpose(
                    ptl[:], xin_sb[:, 128 : 128 + LO + 1], identb[0:E, 0:E]
                )
                nc.scalar.copy(xlv[:, :, bb], ptl[0:LO, :])
                if b == HB - 1:
                    emit_cc1(0)

        # xinp cols: (h, el, c, bb) -- MLP rows follow the same order, so the
        # L1 matmuls for half 0 can run while phase A / cc1b still execute
        xinp_hi_h = [
            perm.tile([128, ROWS // 2], BF16, name=f"xph{h}") for h in range(2)
        ]
        xinp_lo_h2 = [
            perm.tile([LO, ROWS // 2], BF16, name=f"xpl{h}") for h in range(2)
        ]

        def emit_cc1_load(h):
            for el in range(EL):
                csl = slice(32 * el, 32 * (el + 1))
                qsl = slice(HB * el, HB * (el + 1))
                eng = nc.gpsimd if el == 0 else nc.sync
                eng2 = nc.gpsimd if el == 0 else nc.scalar
                eng.dma_start(
                    xinp_hi_h[h][:, csl].rearrange("p (c b) -> p c b", c=NCORES),
                    cc1_out[h][:, 0:128, qsl].rearrange("c d b -> d c b"),
                )
                eng2.dma_start(
                    xinp_lo_h2[h][:, csl].rearrange("p (c b) -> p c b", c=NCORES),
                    cc1_out[h][:, 128 : 128 + LO, qsl].rearrange("c d b -> d c b"),
                )
            # r for this half: 1/s from the s row of xinp_lo
            nc.vector.tensor_copy(
                r_row[0:1, 64 * h : 64 * (h + 1)], xinp_lo_h2[h][LO - 1 : LO, :]
            )
            nc.vector.reciprocal(
                r_row[0:1, 64 * h : 64 * (h + 1)], r_row[0:1, 64 * h : 64 * (h + 1)]
            )
            nc.gpsimd.dma_start(
                r_col[64 * h : 64 * (h + 1), :], r_row[0:1, 64 * h : 64 * (h + 1)]
            )

        # r = 1/s per MLP row (h, el, c, bb); s sits in xinp_lo row LO-1
        r_row = perm.tile([1, ROWS], F32)
        r_col = perm.tile([128, 1], F32)

        # ---------------- MLP (expert-parallel, rows = (el, c, b)) ---------
        def ln_mish(hs, H, gr, ber):
            """LayerNorm + mish of SBUF [128, H] f32 -> bf16.

            Uses only Exp/Ln/Square activations (one act table, no reload):
              rstd = exp(-0.5 * ln(var + eps))
              mish(x) = x * tanh(ln(u)), u = 1 + e^x
                      = x * (1 - 2 * exp(-ln(u^2 + 1)))
            """
            stats = pm.tile([128, 6], F32, tag="stats")
            nc.vector.bn_stats(stats[:], hs)
            mv = pm.tile([128, 2], F32, tag="mv")
            nc.vector.bn_aggr(mv[:], stats[:])
            lnv = pm.tile([128, 1], F32, tag="lnv")
            nc.scalar.activation(lnv[:], mv[:, 1:2], AF.Ln, bias=eps_col[:])
            rstd = pm.tile([128, 1], F32, tag="rstd")
            nc.scalar.activation(rstd[:], lnv[:], AF.Exp, scale=-0.5)
            xn = pm.tile([128, H], F32, tag="xn")
            nc.vector.tensor_scalar(
                xn[:], hs, mv[:, 0:1], rstd[:],
                mybir.AluOpType.subtract, mybir.AluOpType.mult,
            )
            if gr is not None:
                xg = pm.tile([128, H], F32, tag="xg")
                nc.vector.tensor_mul(xg[:], xn[:], gr)
                xn = pm.tile([128, H], F32, tag="xb")
                nc.vector.tensor_add(xn[:], xg[:], ber)
            ex = pm.tile([128, H], F32, tag="ex")
            nc.scalar.activation(ex[:], xn[:], AF.Exp)
            sq2 = pm.tile([128, H], F32, tag="sq2")
            nc.scalar.activation(sq2[:], ex[:], AF.Square, bias=1.0)
            ln2 = pm.tile([128, H], F32, tag="ln2")
            nc.scalar.activation(ln2[:], sq2[:], AF.Ln, bias=1.0)
            wv = pm.tile([128, H], F32, tag="wv")
            nc.scalar.activation(wv[:], ln2[:], AF.Exp, scale=-1.0)
            m = pm.tile([128, H], F32, tag="m")
            nc.vector.tensor_scalar(
                m[:], wv[:], -2.0, 1.0, mybir.AluOpType.mult, mybir.AluOpType.add
            )
            hm = pm.tile([128, H], BF16, tag="hm")
            nc.vector.tensor_mul(hm[:], xn[:], m[:])
            return hm

        w1h_v = w1h_sb[:].rearrange("p (el h) -> p el h", el=EL)
        w1l_v = w1l_sb[:].rearrange("p (el h) -> p el h", el=EL)
        w2_v = w2_sb[:].rearrange("p (el c h) -> p el c h", el=EL, c=C1)
        w3_v = w3_sb[:].rearrange("p (el c h) -> p el c h", el=EL, c=C2)

        with tc.tile_pool(name="ps_mlp", bufs=2, space="PSUM") as ps_mlp, tc.tile_pool(
            name="ps_eo", bufs=1, space="PSUM"
        ) as ps_eo:
            h1ps = ps_mlp.tile([128, H1], F32, tag="h12")

            def emit_l1(h):
                for el in range(EL):
                    osl = slice(64 * h + 32 * el, 64 * h + 32 * (el + 1))
                    csl = slice(32 * el, 32 * (el + 1))
                    tp = (0, 64 * h + 32 * el)
                    nc.tensor.matmul(
                        h1ps[osl, :],
                        xinp_hi_h[h][:, csl],
                        w1h_v[:, el, :],
                        start=True,
                        stop=False,
                        tile_position=tp,
                    )
                    nc.tensor.matmul(
                        h1ps[osl, :],
                        xinp_lo_h2[h][:, csl],
                        w1l_v[:, el, :],
                        start=False,
                        stop=True,
                        tile_position=tp,
                    )

            # half 0: loads + L1 while cc1b is still in flight
            emit_cc1_load(0)
            emit_l1(0)
            emit_cc1(1)
            emit_cc1_load(1)
            emit_l1(1)
            h1s = pm.tile([128, H1], F32, tag="h1s")
            nc.vector.tensor_scalar_mul(h1s[:], h1ps[:], r_col[:])
            h1m = ln_mish(h1s[:], H1, g1_sb[:] if has_g1 else None, be1_sb[:] if has_g1 else None)
            h1T = pm.tile([128, C1 * 128], BF16, tag="h1T")
            nc.sync.dma_start_transpose(
                h1T[:].rearrange("p (c m) -> p c m", c=C1), h1m[:]
            )

            h2ps = ps_mlp.tile([128, H2], F32, tag="h12")
            for h in range(2):
                for el in range(EL):
                    osl = slice(64 * h + 32 * el, 64 * h + 32 * (el + 1))
                    tp = (0, 64 * h + 32 * el)
                    for c in range(C1):
                        nc.tensor.matmul(
                            h2ps[osl, :],
                            h1T[:, c * 128 + 64 * h + 32 * el : c * 128 + 64 * h + 32 * (el + 1)],
                            w2_v[:, el, c, :],
                            start=(c == 0),
                            stop=(c == C1 - 1 and not has_b2),
                            tile_position=tp,
                        )
                    if has_b2:
                        nc.tensor.matmul(
                            h2ps[osl, :],
                            ones_sb[0:1, 0:32],
                            b2_sb[0:1, el * H2 : (el + 1) * H2],
                            start=False,
                            stop=True,
                            tile_position=tp,
                        )
            h2s = pm.tile([128, H2], F32, tag="h2s")
            nc.vector.tensor_copy(h2s[:], h2ps[:])
            h2m = ln_mish(h2s[:], H2, g2_sb[:] if has_g2 else None, be2_sb[:] if has_g2 else None)
            h2T = pm.tile([128, C2 * 128], BF16, tag="h2T")
            nc.sync.dma_start_transpose(
                h2T[:].rearrange("p (c m) -> p c m", c=C2), h2m[:]
            )

            eops = ps_eo.tile([128, DZ], F32, tag="eo")
            for h in range(2):
                for el in range(EL):
                    osl = slice(64 * h + 32 * el, 64 * h + 32 * (el + 1))
                    tp = (0, 64 * h + 32 * el)
                    for c in range(C2):
                        nc.tensor.matmul(
                            eops[osl, :],
                            h2T[:, c * 128 + 64 * h + 32 * el : c * 128 + 64 * h + 32 * (el + 1)],
                            w3_v[:, el, c, :],
                            start=(c == 0),
                            stop=(c == C2 - 1 and not has_b3),
                            tile_position=tp,
                        )
                    if has_b3:
                        nc.tensor.matmul(
                            eops[osl, :],
                            ones_sb[0:1, 0:32],
                            b3_sb[0:1, el * DZ : (el + 1) * DZ],
                            start=False,
                            stop=True,
                            tile_position=tp,
                        )
            eo_sb = pm.tile([128, DZ], BF16, tag="eo_sb")
            nc.vector.tensor_copy(eo_sb[:], eops[:])

        # ---------------- AllToAll 2: expert outputs back ------------------
        cc2_in = dram.tile([NCORES, EL, BC, DZ], BF16)
        cc2_out = dram.tile([NCORES, EL, BC, DZ], BF16)
        for h in range(2):
            for el in range(EL):
                eng = (nc.gpsimd, nc.scalar, nc.sync, nc.gpsimd)[2 * h + el]
                eng.dma_start(
                    cc2_in[:, el, HB * h : HB * (h + 1), :],
                    eo_sb[64 * h + 32 * el : 64 * h + 32 * (el + 1), :],
                )
        nc.gpsimd.collective_compute(
            "AllToAll",
            mybir.AluOpType.bypass,
            replica_groups=[list(range(NCORES))],
            ins=[cc2_in[:].opt()],
            outs=[cc2_out[:].opt()],
        )
        eo_b = [perm.tile([E, DZ], BF16, name=f"eo{b}") for b in range(BC)]
        for b in range(BC):
            eng = (nc.gpsimd, nc.scalar, nc.sync)[b % 3]
            eng.dma_start(eo_b[b][:], cc2_out[:, :, b, :])

        # ---------------- Combine (data-parallel) --------------------------
        with tc.tile_pool(name="ps_cmb", bufs=4, space="PSUM") as ps_cmb:
            for b in range(BC):
                osb = posb.tile([128, NT * DZ], F32, tag="osb")
                ov = osb[:].rearrange("p (t d) -> p t d", d=DZ)
                for t in range(NT):
                    cps = ps_cmb.tile([128, DZ], F32, tag="c")
                    nc.tensor.matmul(
                        cps[:],
                        expCT[b][:, 128 * t : 128 * (t + 1)],
                        eo_b[b][:],
                        start=True,
                        stop=True,
                    )
                    eng = (nc.vector, nc.scalar)[t % 2]
                    if eng is nc.scalar:
                        eng.mul(ov[:, t, :], cps[:], rcomb[b][:, t : t + 1])
                    else:
                        eng.tensor_scalar_mul(
                            ov[:, t, :], cps[:], rcomb[b][:, t : t + 1]
                        )
                eng = nc.gpsimd if b % 2 == 0 else nc.sync
                eng.dma_start(out[b], ov[:, :, :])
    return nc


# ---------------------------------------------------------------------------
# Host wrapper
# ---------------------------------------------------------------------------

_CACHE = {}


def _get_nc(key, *args):
    if key not in _CACHE:
        _CACHE[key] = build_nc(*args)
    return _CACHE[key]


def _prepare(z, a, phi, W1, b1, g1, be1, W2, b2, g2, be2, W3, b3):
    """Build (cached) the Bass program and per-core input maps."""
    import ml_dtypes

    BF = ml_dtypes.bfloat16

    z = np.asarray(z, np.float32)
    a = np.asarray(a, np.float32)
    phi = np.asarray(phi, np.float32)
    W1 = np.asarray(W1, np.float32)
    b1 = np.asarray(b1, np.float32)
    g1 = np.asarray(g1, np.float32)
    be1 = np.asarray(be1, np.float32)
    W2 = np.asarray(W2, np.float32)
    b2 = np.asarray(b2, np.float32)
    g2 = np.asarray(g2, np.float32)
    be2 = np.asarray(be2, np.float32)
    W3 = np.asarray(W3, np.float32)
    b3 = np.asarray(b3, np.float32)

    B, N, DZ = z.shape
    DA = a.shape[2]
    D = DZ + DA
    E = W1.shape[0]
    H1 = W1.shape[2]
    H2 = W2.shape[2]
    BC = B // NCORES
    EL = E // NCORES
    NT = N // 128
    XA = D + 4

    has_b2 = bool(np.any(b2))
    has_b3 = bool(np.any(b3))
    has_g1 = not (np.all(g1 == 1.0) and np.all(be1 == 0.0))
    has_g2 = not (np.all(g2 == 1.0) and np.all(be2 == 0.0))

    key = (BC, N, DZ, DA, E, H1, H2, has_b2, has_b3, has_g1, has_g2)
    nc = _get_nc(key, *key)

    # x native, padded with ones: [B, N, XA] -> [B, 128, NT, XA]
    xf = np.empty((B, N, XA), np.float32)
    xf[:, :, 0:DZ] = z
    xf[:, :, DZ:D] = a
    xf[:, :, D:XA] = 1.0
    xa_all = np.ascontiguousarray(
        xf.reshape(B, 128, NT, XA)
    ).astype(BF)  # token n = p*NT + t
    # x transposed, tile-shuffled: [B, D, NT, 128] with col (t, p)
    xT = np.concatenate([z, a], axis=-1).transpose(0, 2, 1)  # [B, D, N]
    xTs = xT.reshape(B, D, 128, NT).transpose(0, 1, 3, 2)  # [B, D, NT, 128]
    zTs_all = np.ascontiguousarray(xTs[:, 0:128]).astype(BF)
    aTs_all = np.ascontiguousarray(xTs[:, 128:D]).astype(BF)

    phi2 = np.ascontiguousarray(phi.reshape(D, -1)).astype(BF)
    w1aug = np.concatenate([W1, b1[:, None, :]], axis=1)  # [E, D+1, H1]
    identb = np.eye(128, dtype=np.float32).astype(BF)

    in_maps = []
    for i in range(NCORES):
        es = slice(i * EL, (i + 1) * EL)
        m = {
            "xa": xa_all[i * BC : (i + 1) * BC],
            "zTs": zTs_all[i * BC : (i + 1) * BC],
            "aTs": aTs_all[i * BC : (i + 1) * BC],
            "phi_d": phi2,
            "w1h_d": np.ascontiguousarray(w1aug[es, 0:128]).astype(BF),
            "w1l_d": np.ascontiguousarray(w1aug[es, 128 : D + 1]).astype(BF),
            "w2_d": np.ascontiguousarray(
                W2[es].reshape(EL, H1 // 128, 128, H2)
            ).astype(BF),
            "w3_d": np.ascontiguousarray(
                W3[es].reshape(EL, H2 // 128, 128, DZ)
            ).astype(BF),
            "identb_d": identb,
        }
        if has_b2:
            m["b2_d"] = np.ascontiguousarray(b2[es].reshape(1, -1)).astype(BF)
            m["ones_d"] = np.ones((1, 128), np.float32).astype(BF)
        if has_b3:
            m["b3_d"] = np.ascontiguousarray(b3[es].reshape(1, -1)).astype(BF)
        if has_g1:
            m["g1_d"] = np.ascontiguousarray(np.tile(np.repeat(g1[es], 32, 0), (2, 1)))
            m["be1_d"] = np.ascontiguousarray(np.tile(np.repeat(be1[es], 32, 0), (2, 1)))
        if has_g2:
            m["g2_d"] = np.ascontiguousarray(np.tile(np.repeat(g2[es], 32, 0), (2, 1)))
            m["be2_d"] = np.ascontiguousarray(np.tile(np.repeat(be2[es], 32, 0), (2, 1)))
        in_maps.append(m)
    return nc, in_maps


def kernel(**inputs):
    nc, in_maps = _prepare(**inputs)

    from concourse.bass_utils import run_bass_kernel_spmd

    res = run_bass_kernel_spmd(nc, in_maps, list(range(NCORES)))
    B = len(in_maps) * in_maps[0]["xa"].shape[0]
    outs = []
    for r in res.results:
        o = r["out"]  # [BC, 128, NT, DZ], token n = p*NT + t
        BC, P, NT_, DZ_ = o.shape
        outs.append(o.reshape(BC, P * NT_, DZ_))
    return np.concatenate(outs, axis=0)


# revision 29
# speedup vs baseline: 1.3160x; 1.0258x over previous
"""Trainium2 Bass kernel for nn_CenMoEDynamicsModel (MoE routing), v2.

Contract: kernel(**inputs) takes FULL unsharded numpy inputs and returns the
FULL [64, 2048, 128] f32 output. Data-parallel over B across 8 NeuronCores for
routing + combine; expert-parallel (2 experts/core) for the MLP with two tiny
AllToAll exchanges (xin out, expert_outputs back).

Math (per batch b):
  x = [z|a]                       [N, D]     D = 192
  w = x @ phi                     [N, E]     E = 16
  dispatch = softmax_n(w); xin = dispatch^T @ x          [E, D]
  h = mish(LN(xin@W1+b1)); h = mish(LN(h@W2+b2)); EO = h@W3+b3   [E, DZ]
  combine = softmax_e(w); out = combine @ EO             [N, DZ]

Key design points (vs v1 baseline):
  - everything bf16 on the PE (1 cyc/row always; f32r pays 4 cyc/row for
    moving dims < 256) and half the HBM/DMA bytes.
  - token-major routing: out[128tok, E] streams only E=16 rows per matmul
    (vs 512 for e-major), and exp() writes the token-major ec tiles the xin
    matmul needs directly. e-major expCT (combine lhsT) comes from 16 cheap
    PE transposes per batch.
  - combine softmax denominators reduced from ec tiles in phase A (DVE),
    so combine is matmul + one scale/evacuation op per tile, no ones column.
  - expert weights sharded: each core loads 2 experts (1.7 MB bf16 vs 27 MB
    f32 for all 16). xin packs are exchanged with AllToAll ([8,193,16] bf16),
    expert outputs return with a second AllToAll ([8,16,128] bf16).
  - MLP activations packed [2 experts x 64 batches, H] = full 128 partitions;
    PE writes both experts into one PSUM tile (offsets 0/64). Inter-layer
    transposes use the DMA crossbar (dma_start_transpose), not the PE.
  - single top-level SBUF pool scope => weight prefetch DMAs (gpsimd queue)
    overlap phase A input streaming (sync queue); no inter-phase SBUF
    anti-dependency barriers.

ones-column tricks kept from v1: 4 ones columns appended to x give the
dispatch denominator s_e as xin row D; W1 is bias-augmented so scaling L1 rows
by r = 1/s normalizes dispatch and bias exactly.
"""

import sys

import numpy as np

sys.path.insert(0, "/opt/trn_rl_repo")

from contextlib import ExitStack

import concourse.bass as bass
import concourse.tile as tile
from concourse import mybir

F32 = mybir.dt.float32
BF16 = mybir.dt.bfloat16
AF = mybir.ActivationFunctionType

LN_EPS = 1e-5
NCORES = 8


def _split_drain_and_barrier(self, tick_clock, wait_clock):
    """Replacement for TileContext._drain_and_barrier.

    The stock version attaches every outstanding semaphore wait to ONE tail
    Drain instruction; this walrus build's codegen rejects Drains with more
    than a couple of sync waits ("Too many sync wait commands"). Emit one
    single-wait Drain per logical proc instead (the SP queue executes them in
    order, so the final bare drain still happens after everything finished).
    """
    from concourse.vector_clock import ScopedClock, VectorClock

    nc = self.nc
    gc = tick_clock.global_clock
    n = len(gc)
    for i in range(n):
        t = gc[i]
        if t <= 0:
            continue
        v = VectorClock([0] * n)
        v.require_at_least(i, t)
        d = nc.sync.drain()
        wait_clock.add_sem_waits(d.ins, ScopedClock({None: v}))
    nc.sync.drain()
    nc.all_engine_barrier()
    assert self.sems is not None
    popped = nc._tile_sem_poison_stack.pop()
    assert popped is self._sem_poison
    nc.clear_and_free_semaphores(list(self.sems.allocated().values()))
    nc.all_engine_barrier()


tile.TileContext._drain_and_barrier = _split_drain_and_barrier

# This walrus build rejects instructions carrying more than a couple of sync
# waits ("Too many sync wait commands" in CoreV3 codegen), while Tile freely
# attaches 3+. Split excess waits onto NoOp carrier instructions (same engine
# queue, executed in order => semantics preserved) at BIR-serialization time.
_MAX_WAITS = 1


def _split_waits_json(bir: bytes) -> bytes:
    import orjson

    m = orjson.loads(bir)
    changed = False
    ctr = 0
    for f in m.get("functions", []):
        for b in f.get("blocks", []):
            out = []
            for i in b.get("instructions", []):
                si = i.get("sync_info")
                ow = (si or {}).get("on_wait") or []
                if len(ow) > _MAX_WAITS:
                    head = ow[: -_MAX_WAITS]
                    for j in range(0, len(head), _MAX_WAITS):
                        ctr += 1
                        out.append(
                            {
                                "debug": i.get("debug", 0),
                                "engine": i["engine"],
                                "ins": [],
                                "outs": [],
                                "name": f"{i['name']}-wsplit{ctr}",
                                "opcode": "NoOp",
                                "sync_info": {
                                    "on_wait": head[j : j + _MAX_WAITS],
                                    "on_update": [],
                                },
                            }
                        )
                    si["on_wait"] = ow[-_MAX_WAITS:]
                    changed = True
                out.append(i)
            b["instructions"] = out
    return orjson.dumps(m) if changed else bir


_orig_to_json_bytes = bass.Bass.to_json_bytes


def _patched_to_json_bytes(self):
    return _split_waits_json(_orig_to_json_bytes(self))


bass.Bass.to_json_bytes = _patched_to_json_bytes


def build_nc(BC, N, DZ, DA, E, H1, H2, has_b2, has_b3, has_g1, has_g2):
    """Build the per-core Bass program (SPMD, rank-free).

    BC batches per core. EL = E // NCORES local experts.
    """
    D = DZ + DA  # 192
    NT = N // 128  # 16 token tiles per batch
    EL = E // NCORES  # 2
    XA = D + 4  # x padded with 4 ones columns (col D = s_e trick)
    C1 = H1 // 128
    C2 = H2 // 128
    ROWS = EL * NCORES * BC  # 128 MLP pack rows = (el, c, b)
    assert DZ == 128 and E == 16 and BC == 8 and EL == 2 and ROWS == 128

    nc = bass.Bass(num_devices=NCORES)
    import os as _os
    for _ in range(int(_os.environ.get("K_SALT", "0"))):
        nc.sync.nop()

    xa = nc.dram_tensor("xa", [BC, 128, NT, XA], BF16, kind="ExternalInput")
    zTs = nc.dram_tensor("zTs", [BC, 128, NT, 128], BF16, kind="ExternalInput")
    aTs = nc.dram_tensor("aTs", [BC, DA, NT, 128], BF16, kind="ExternalInput")
    phi_d = nc.dram_tensor("phi_d", [D, E], BF16, kind="ExternalInput")
    w1h_d = nc.dram_tensor("w1h_d", [EL, 128, H1], BF16, kind="ExternalInput")
    w1l_d = nc.dram_tensor("w1l_d", [EL, D - 128 + 1, H1], BF16, kind="ExternalInput")
    w2_d = nc.dram_tensor("w2_d", [EL, C1, 128, H2], BF16, kind="ExternalInput")
    w3_d = nc.dram_tensor("w3_d", [EL, C2, 128, DZ], BF16, kind="ExternalInput")
    identb_d = nc.dram_tensor("identb_d", [128, 128], BF16, kind="ExternalInput")
    if has_b2:
        b2_d = nc.dram_tensor("b2_d", [1, EL * H2], BF16, kind="ExternalInput")
        ones_d = nc.dram_tensor("ones_d", [1, ROWS], BF16, kind="ExternalInput")
    if has_b3:
        b3_d = nc.dram_tensor("b3_d", [1, EL * DZ], BF16, kind="ExternalInput")
    if has_g1:
        g1_d = nc.dram_tensor("g1_d", [ROWS, H1], F32, kind="ExternalInput")
        be1_d = nc.dram_tensor("be1_d", [ROWS, H1], F32, kind="ExternalInput")
    if has_g2:
        g2_d = nc.dram_tensor("g2_d", [ROWS, H2], F32, kind="ExternalInput")
        be2_d = nc.dram_tensor("be2_d", [ROWS, H2], F32, kind="ExternalInput")
    out = nc.dram_tensor("out", [BC, 128, NT, DZ], F32, kind="ExternalOutput")

    LO = D - 128 + 1  # 65 rows: a-features 128..191 plus the s row

    with tile.TileContext(nc) as tc, ExitStack() as ctx:
        perm = ctx.enter_context(tc.tile_pool(name="perm", bufs=1))
        pa = ctx.enter_context(tc.tile_pool(name="pa", bufs=2))
        pec = ctx.enter_context(tc.tile_pool(name="pec", bufs=4))
        pm = ctx.enter_context(tc.tile_pool(name="pm", bufs=1))
        pst = ctx.enter_context(tc.tile_pool(name="pst", bufs=2))
        posb = ctx.enter_context(tc.tile_pool(name="posb", bufs=2))
        dram = ctx.enter_context(tc.tile_pool(name="dram", bufs=1, space="DRAM"))

        # ---------------- weight / constant prefetch (gpsimd queue) --------
        identb = perm.tile([128, 128], BF16)
        nc.gpsimd.dma_start(identb[:], identb_d[:, :])
        phi_hi = perm.tile([128, E], BF16)
        phi_lo = perm.tile([DA, E], BF16)
        nc.gpsimd.dma_start(phi_hi[:], phi_d[0:128, :])
        nc.gpsimd.dma_start(phi_lo[:], phi_d[128:D, :])
        w1h_sb = perm.tile([128, EL * H1], BF16)
        w1l_sb = perm.tile([LO, EL * H1], BF16)
        w2_sb = perm.tile([128, EL * C1 * H2], BF16)
        w3_sb = perm.tile([128, EL * C2 * DZ], BF16)
        nc.gpsimd.dma_start(
            w1h_sb[:].rearrange("p (el h) -> p el h", el=EL), w1h_d[:, :, :].rearrange("el p h -> p el h")
        )
        nc.gpsimd.dma_start(
            w1l_sb[:].rearrange("p (el h) -> p el h", el=EL), w1l_d[:, :, :].rearrange("el p h -> p el h")
        )
        nc.gpsimd.dma_start(
            w2_sb[:].rearrange("p (el c h) -> p el c h", el=EL, c=C1),
            w2_d[:, :, :, :].rearrange("el c p h -> p el c h"),
        )
        nc.gpsimd.dma_start(
            w3_sb[:].rearrange("p (el c h) -> p el c h", el=EL, c=C2),
            w3_d[:, :, :, :].rearrange("el c p h -> p el c h"),
        )
        if has_b2:
            b2_sb = perm.tile([1, EL * H2], BF16)
            nc.gpsimd.dma_start(b2_sb[:], b2_d[:, :])
            ones_sb = perm.tile([1, ROWS], BF16)
            nc.gpsimd.dma_start(ones_sb[:], ones_d[:, :])
        if has_b3:
            b3_sb = perm.tile([1, EL * DZ], BF16)
            nc.gpsimd.dma_start(b3_sb[:], b3_d[:, :])
        g1_sb = be1_sb = g2_sb = be2_sb = None
        if has_g1:
            g1_sb = perm.tile([ROWS, H1], F32)
            be1_sb = perm.tile([ROWS, H1], F32)
            nc.gpsimd.dma_start(g1_sb[:], g1_d[:, :])
            nc.gpsimd.dma_start(be1_sb[:], be1_d[:, :])
        if has_g2:
            g2_sb = perm.tile([ROWS, H2], F32)
            be2_sb = perm.tile([ROWS, H2], F32)
            nc.gpsimd.dma_start(g2_sb[:], g2_d[:, :])
            nc.gpsimd.dma_start(be2_sb[:], be2_d[:, :])

        # dummy collective: absorbs the first-collective warmup latency (the
        # first CC on this fabric costs ~15us extra) while phase A computes
        ccw_in = dram.tile([NCORES, 2], F32, name="ccw_in")
        ccw_out = dram.tile([NCORES, 2], F32, name="ccw_out")
        ccw_sb = perm.tile([NCORES, 2], F32)
        nc.gpsimd.memset(ccw_sb[:], 0.0)
        nc.gpsimd.dma_start(ccw_in[:], ccw_sb[:])
        for _ in range(2):
            nc.gpsimd.collective_compute(
                "AllToAll",
                mybir.AluOpType.bypass,
                replica_groups=[list(range(NCORES))],
                ins=[ccw_in[:].opt()],
                outs=[ccw_out[:].opt()],
            )

        # persistent phase-A outputs
        expCT = [perm.tile([E, N], BF16, name=f"expCT{b}") for b in range(BC)]
        rcomb = [perm.tile([128, NT], F32, name=f"rcomb{b}") for b in range(BC)]
        # xin packs split into batch halves so the first AllToAll can launch
        # while phase A still works on batches BC/2..BC-1
        HB = BC // 2
        xin_hi_h = [perm.tile([128, E * HB], BF16, name=f"xinh{h}") for h in range(2)]
        xin_lo_h = [perm.tile([LO, E * HB], BF16, name=f"xinl{h}") for h in range(2)]
        eps_col = perm.tile([128, 1], F32)
        nc.vector.memset(eps_col[:], LN_EPS)

        cc1_in = [
            dram.tile([NCORES, 128 + LO, EL * HB], BF16, name=f"cc1_in{h}")
            for h in range(2)
        ]
        cc1_out = [
            dram.tile([NCORES, 128 + LO, EL * HB], BF16, name=f"cc1_out{h}")
            for h in range(2)
        ]

        def emit_cc1(h):
            nc.gpsimd.dma_start(
                cc1_in[h][:, 0:128, :].rearrange("j d q -> d j q"),
                xin_hi_h[h][:].rearrange("p (j q) -> p j q", j=NCORES),
            )
            nc.gpsimd.dma_start(
                cc1_in[h][:, 128 : 128 + LO, :].rearrange("j d q -> d j q"),
                xin_lo_h[h][:].rearrange("p (j q) -> p j q", j=NCORES),
            )
            nc.gpsimd.collective_compute(
                "AllToAll",
                mybir.AluOpType.bypass,
                replica_groups=[list(range(NCORES))],
                ins=[cc1_in[h][:].opt()],
                outs=[cc1_out[h][:].opt()],
            )

        # ---------------- Phase A: routing + xin (data-parallel) -----------
        with tc.tile_pool(name="ps_wq", bufs=2, space="PSUM") as ps_wq, tc.tile_pool(
            name="ps_ect", bufs=2, space="PSUM"
        ) as ps_ect, tc.tile_pool(name="ps_xin", bufs=2, space="PSUM") as ps_xin, tc.tile_pool(
            name="ps_xtr", bufs=1, space="PSUM"
        ) as ps_xtr:
            for b in range(BC):
                zT_sb = pa.tile([128, NT * 128], BF16, tag="zT")
                aT_sb = pa.tile([DA, NT * 128], BF16, tag="aT")
                nc.sync.dma_start(
                    zT_sb[:].rearrange("p (t c) -> p t c", c=128), zTs[b]
                )
                nc.sync.dma_start(
                    aT_sb[:].rearrange("p (t c) -> p t c", c=128), aTs[b]
                )
                x_sb = pa.tile([128, NT * XA], BF16, tag="x")
                xv = x_sb[:].rearrange("p (t c) -> p t c", c=XA)
                nc.sync.dma_start(xv[:, :, :], xa[b])
                zTv = zT_sb[:].rearrange("p (t c) -> p t c", c=128)
                aTv = aT_sb[:].rearrange("p (t c) -> p t c", c=128)

                xinps = ps_xin.tile([E, XA], F32, tag="xin")
                denom = pa.tile([128, NT], F32, tag="denom")
                for g in range(NT // 4):
                    # e-major routing: one [16, 512] chunk covers 4 token
                    # tiles; exp writes the combine lhsT (expCT) directly
                    sl512 = slice(512 * g, 512 * (g + 1))
                    wqe = ps_wq.tile([E, 512], F32, tag="wq")
                    nc.tensor.matmul(
                        wqe[:], phi_hi[:], zT_sb[:, sl512], start=True, stop=False
                    )
                    nc.tensor.matmul(
                        wqe[:], phi_lo[:], aT_sb[:, sl512], start=False, stop=True
                    )
                    nc.scalar.activation(expCT[b][:, sl512], wqe[:], AF.Exp)
                    # token-major ec tiles for the xin contraction
                    ec4 = pec.tile([128, 64], BF16, tag="ec")
                    for j in range(4):
                        t = g * 4 + j
                        sl = slice(16 * j, 16 * (j + 1))
                        ecp = ps_ect.tile([128, E], BF16, tag="ect")
                        nc.tensor.transpose(
                            ecp[:],
                            expCT[b][:, 128 * t : 128 * (t + 1)],
                            identb[0:E, 0:E],
                        )
                        eng = nc.vector if j % 2 == 0 else nc.scalar
                        cpy = eng.copy if eng is nc.scalar else eng.tensor_copy
                        cpy(ec4[:, sl], ecp[:])
                        # xin accumulation (contract tokens)
                        nc.tensor.matmul(
                            xinps[:],
                            ec4[:, sl],
                            xv[:, t, :],
                            start=(t == 0),
                            stop=(t == NT - 1),
                        )
                    # combine denominators for these 4 tiles
                    nc.vector.tensor_reduce(
                        denom[:, 4 * g : 4 * (g + 1)],
                        ec4[:].rearrange("p (u e) -> p u e", u=4),
                        mybir.AxisListType.X,
                        mybir.AluOpType.add,
                    )
                # reciprocal of combine denominators (all 16 tiles at once)
                nc.vector.reciprocal(rcomb[b][:], denom[:])
                # xin -> bf16 -> transposed into the [d, (e b-half)] packs
                xin_sb = pec.tile([E, XA], BF16, tag="xin_sb")
                nc.scalar.copy(xin_sb[:], xinps[:])
                h, bb = b // HB, b % HB
                xhv = xin_hi_h[h][:].rearrange("p (e b) -> p e b", b=HB)
                xlv = xin_lo_h[h][:].rearrange("p (e b) -> p e b", b=HB)
                pth = ps_xtr.tile([128, E], BF16, tag="trh")
                nc.tensor.transpose(pth[:], xin_sb[:, 0:128], identb[0:E, 0:E])
                nc.vector.tensor_copy(xhv[:, :, bb], pth[:])
                ptl = ps_xtr.tile([LO + 1, E], BF16, tag="trl")
                nc.tensor.transpose(
                    ptl[:], xin_sb[:, 128 : 128 + LO + 1], identb[0:E, 0:E]
                )
                nc.scalar.copy(xlv[:, :, bb], ptl[0:LO, :])
                if b == HB - 1:
                    emit_cc1(0)

        # xinp cols: (h, el, c, bb) -- MLP rows follow the same order, so the
        # L1 matmuls for half 0 can run while phase A / cc1b still execute
        xinp_hi_h = [
            perm.tile([128, ROWS // 2], BF16, name=f"xph{h}") for h in range(2)
        ]
        xinp_lo_h2 = [
            perm.tile([LO, ROWS // 2], BF16, name=f"xpl{h}") for h in range(2)
        ]

        def emit_cc1_load(h):
            for el in range(EL):
                csl = slice(32 * el, 32 * (el + 1))
                qsl = slice(HB * el, HB * (el + 1))
                nc.gpsimd.dma_start(
                    xinp_hi_h[h][:, csl].rearrange("p (c b) -> p c b", c=NCORES),
                    cc1_out[h][:, 0:128, qsl].rearrange("c d b -> d c b"),
                )
                nc.gpsimd.dma_start(
                    xinp_lo_h2[h][:, csl].rearrange("p (c b) -> p c b", c=NCORES),
                    cc1_out[h][:, 128 : 128 + LO, qsl].rearrange("c d b -> d c b"),
                )
            # r for this half: 1/s from the s row of xinp_lo
            nc.vector.tensor_copy(
                r_row[0:1, 64 * h : 64 * (h + 1)], xinp_lo_h2[h][LO - 1 : LO, :]
            )
            nc.vector.reciprocal(
                r_row[0:1, 64 * h : 64 * (h + 1)], r_row[0:1, 64 * h : 64 * (h + 1)]
            )
            nc.gpsimd.dma_start(
                r_col[64 * h : 64 * (h + 1), :], r_row[0:1, 64 * h : 64 * (h + 1)]
            )

        # r = 1/s per MLP row (h, el, c, bb); s sits in xinp_lo row LO-1
        r_row = perm.tile([1, ROWS], F32)
        r_col = perm.tile([128, 1], F32)
        eo_b = [perm.tile([E, DZ], BF16, name=f"eo{b}") for b in range(BC)]

        # ------- MLP + combine, split into two batch-half pipelines --------
        # Half h covers batches h*4..h*4+3 of every core; rows = (el, c, bb).
        # All half tensors live at base partition 0 with 64 rows.
        def ln_mish(hs, H, gr, ber, tg):
            """LayerNorm + mish of SBUF [64, H] f32 -> bf16.

            Uses only Exp/Ln/Square activations (one act table, no reload):
              rstd = exp(-0.5 * ln(var + eps))
              mish(x) = x * tanh(ln(u)) = x * (1 - 2*exp(-ln(u^2+1))), u = 1+e^x
            """
            stats = pm.tile([64, 6], F32, tag=f"stats{tg}")
            nc.vector.bn_stats(stats[:], hs)
            mv = pm.tile([64, 2], F32, tag=f"mv{tg}")
            nc.vector.bn_aggr(mv[:], stats[:])
            lnv = pm.tile([64, 1], F32, tag=f"lnv{tg}")
            nc.scalar.activation(lnv[:], mv[:, 1:2], AF.Ln, bias=eps_col[0:64, :])
            rstd = pm.tile([64, 1], F32, tag=f"rstd{tg}")
            nc.scalar.activation(rstd[:], lnv[:], AF.Exp, scale=-0.5)
            xn = pm.tile([64, H], F32, tag=f"xn{tg}")
            nc.vector.tensor_scalar(
                xn[:], hs, mv[:, 0:1], rstd[:],
                mybir.AluOpType.subtract, mybir.AluOpType.mult,
            )
            if gr is not None:
                xg = pm.tile([64, H], F32, tag=f"xg{tg}")
                nc.vector.tensor_mul(xg[:], xn[:], gr)
                xn = pm.tile([64, H], F32, tag=f"xb{tg}")
                nc.vector.tensor_add(xn[:], xg[:], ber)
            ex = pm.tile([64, H], F32, tag=f"ex{tg}")
            nc.scalar.activation(ex[:], xn[:], AF.Exp)
            sq2 = pm.tile([64, H], F32, tag=f"sq2{tg}")
            nc.scalar.activation(sq2[:], ex[:], AF.Square, bias=1.0)
            ln2 = pm.tile([64, H], F32, tag=f"ln2{tg}")
            nc.scalar.activation(ln2[:], sq2[:], AF.Ln, bias=1.0)
            wv = pm.tile([64, H], F32, tag=f"wv{tg}")
            nc.scalar.activation(wv[:], ln2[:], AF.Exp, scale=-1.0)
            m = pm.tile([64, H], F32, tag=f"m{tg}")
            nc.vector.tensor_scalar(
                m[:], wv[:], -2.0, 1.0, mybir.AluOpType.mult, mybir.AluOpType.add
            )
            hm = pm.tile([64, H], BF16, tag=f"hm{tg}")
            nc.vector.tensor_mul(hm[:], xn[:], m[:])
            return hm

        w1h_v = w1h_sb[:].rearrange("p (el h) -> p el h", el=EL)
        w1l_v = w1l_sb[:].rearrange("p (el h) -> p el h", el=EL)
        w2_v = w2_sb[:].rearrange("p (el c h) -> p el c h", el=EL, c=C1)
        w3_v = w3_sb[:].rearrange("p (el c h) -> p el c h", el=EL, c=C2)

        with tc.tile_pool(name="ps_mlp", bufs=3, space="PSUM") as ps_mlp, tc.tile_pool(
            name="ps_cmb", bufs=4, space="PSUM"
        ) as ps_cmb:

            def mlp_half(h):
                rsl = slice(64 * h, 64 * (h + 1))
                h1ps = ps_mlp.tile([64, H1], F32, tag="ps64", name=f"h1ps{h}")
                for el in range(EL):
                    osl = slice(32 * el, 32 * (el + 1))
                    csl = slice(32 * el, 32 * (el + 1))
                    tp = (0, 32 * el)
                    nc.tensor.matmul(
                        h1ps[osl, :], xinp_hi_h[h][:, csl], w1h_v[:, el, :],
                        start=True, stop=False, tile_position=tp,
                    )
                    nc.tensor.matmul(
                        h1ps[osl, :], xinp_lo_h2[h][:, csl], w1l_v[:, el, :],
                        start=False, stop=True, tile_position=tp,
                    )
                h1s = pm.tile([64, H1], F32, tag=f"h1s{h}")
                nc.vector.tensor_scalar_mul(h1s[:], h1ps[:], r_col[rsl, :])
                h1m = ln_mish(
                    h1s[:], H1,
                    g1_sb[rsl, :] if has_g1 else None,
                    be1_sb[rsl, :] if has_g1 else None,
                    f"1{h}",
                )
                h1T = pm.tile([128, C1 * 64], BF16, tag=f"h1T{h}")
                nc.sync.dma_start_transpose(
                    h1T[:].rearrange("p (c m) -> p c m", c=C1), h1m[:]
                )
                h2ps = ps_mlp.tile([64, H2], F32, tag="ps64", name=f"h2ps{h}")
                for el in range(EL):
                    osl = slice(32 * el, 32 * (el + 1))
                    tp = (0, 32 * el)
                    for c in range(C1):
                        nc.tensor.matmul(
                            h2ps[osl, :],
                            h1T[:, c * 64 + 32 * el : c * 64 + 32 * (el + 1)],
                            w2_v[:, el, c, :],
                            start=(c == 0),
                            stop=(c == C1 - 1 and not has_b2),
                            tile_position=tp,
                        )
                    if has_b2:
                        nc.tensor.matmul(
                            h2ps[osl, :],
                            ones_sb[0:1, 0:32],
                            b2_sb[0:1, el * H2 : (el + 1) * H2],
                            start=False, stop=True, tile_position=tp,
                        )
                h2s = pm.tile([64, H2], F32, tag=f"h2s{h}")
                nc.vector.tensor_copy(h2s[:], h2ps[:])
                h2m = ln_mish(
                    h2s[:], H2,
                    g2_sb[rsl, :] if has_g2 else None,
                    be2_sb[rsl, :] if has_g2 else None,
                    f"2{h}",
                )
                h2T = pm.tile([128, C2 * 64], BF16, tag=f"h2T{h}")
                nc.sync.dma_start_transpose(
                    h2T[:].rearrange("p (c m) -> p c m", c=C2), h2m[:]
                )
                eops = ps_mlp.tile([64, H2], F32, tag="ps64", name=f"eops{h}")
                for el in range(EL):
                    osl = slice(32 * el, 32 * (el + 1))
                    tp = (0, 32 * el)
                    for c in range(C2):
                        nc.tensor.matmul(
                            eops[osl, 0:DZ],
                            h2T[:, c * 64 + 32 * el : c * 64 + 32 * (el + 1)],
                            w3_v[:, el, c, :],
                            start=(c == 0),
                            stop=(c == C2 - 1 and not has_b3),
                            tile_position=tp,
                        )
                    if has_b3:
                        nc.tensor.matmul(
                            eops[osl, 0:DZ],
                            ones_sb[0:1, 0:32],
                            b3_sb[0:1, el * DZ : (el + 1) * DZ],
                            start=False, stop=True, tile_position=tp,
                        )
                eo_sbh = pm.tile([64, DZ], BF16, tag=f"eosb{h}")
                nc.vector.tensor_copy(eo_sbh[:], eops[:, 0:DZ])
                return eo_sbh

            def cc2_half(h, eo_sbh):
                cc2h_in = dram.tile([NCORES, EL, HB, DZ], BF16, name=f"cc2i{h}")
                cc2h_out = dram.tile([NCORES, EL, HB, DZ], BF16, name=f"cc2o{h}")
                for el in range(EL):
                    nc.gpsimd.dma_start(
                        cc2h_in[:, el, :, :], eo_sbh[32 * el : 32 * (el + 1), :]
                    )
                nc.gpsimd.collective_compute(
                    "AllToAll",
                    mybir.AluOpType.bypass,
                    replica_groups=[list(range(NCORES))],
                    ins=[cc2h_in[:].opt()],
                    outs=[cc2h_out[:].opt()],
                )
                for bb in range(HB):
                    nc.gpsimd.dma_start(
                        eo_b[h * HB + bb][:], cc2h_out[:, :, bb, :]
                    )

            def combine_half(h):
                for bb in range(HB):
                    b = h * HB + bb
                    osb = posb.tile([128, NT * DZ], F32, tag="osb")
                    ov = osb[:].rearrange("p (t d) -> p t d", d=DZ)
                    for t in range(NT):
                        cps = ps_cmb.tile([128, DZ], F32, tag="c")
                        nc.tensor.matmul(
                            cps[:],
                            expCT[b][:, 128 * t : 128 * (t + 1)],
                            eo_b[b][:],
                            start=True,
                            stop=True,
                        )
                        eng = (nc.vector, nc.scalar)[t % 2]
                        if eng is nc.scalar:
                            eng.mul(ov[:, t, :], cps[:], rcomb[b][:, t : t + 1])
                        else:
                            eng.tensor_scalar_mul(
                                ov[:, t, :], cps[:], rcomb[b][:, t : t + 1]
                            )
                    nc.sync.dma_start(out[b], ov[:, :, :])

            # orchestration: half-0 exchange + MLP run in the shadow of the
            # second collective; combine h0 overlaps MLP h1 / cc2b
            emit_cc1_load(0)
            emit_cc1(1)
            emit_cc1_load(1)
            eo0 = mlp_half(0)
            cc2_half(0, eo0)
            eo1 = mlp_half(1)
            cc2_half(1, eo1)
            combine_half(0)
            combine_half(1)
    return nc


# ---------------------------------------------------------------------------
# Host wrapper
# ---------------------------------------------------------------------------

_CACHE = {}


def _get_nc(key, *args):
    if key not in _CACHE:
        _CACHE[key] = build_nc(*args)
    return _CACHE[key]


def _prepare(z, a, phi, W1, b1, g1, be1, W2, b2, g2, be2, W3, b3):
    """Build (cached) the Bass program and per-core input maps."""
    import ml_dtypes

    BF = ml_dtypes.bfloat16

    z = np.asarray(z, np.float32)
    a = np.asarray(a, np.float32)
    phi = np.asarray(phi, np.float32)
    W1 = np.asarray(W1, np.float32)
    b1 = np.asarray(b1, np.float32)
    g1 = np.asarray(g1, np.float32)
    be1 = np.asarray(be1, np.float32)
    W2 = np.asarray(W2, np.float32)
    b2 = np.asarray(b2, np.float32)
    g2 = np.asarray(g2, np.float32)
    be2 = np.asarray(be2, np.float32)
    W3 = np.asarray(W3, np.float32)
    b3 = np.asarray(b3, np.float32)

    B, N, DZ = z.shape
    DA = a.shape[2]
    D = DZ + DA
    E = W1.shape[0]
    H1 = W1.shape[2]
    H2 = W2.shape[2]
    BC = B // NCORES
    EL = E // NCORES
    NT = N // 128
    XA = D + 4

    has_b2 = bool(np.any(b2))
    has_b3 = bool(np.any(b3))
    has_g1 = not (np.all(g1 == 1.0) and np.all(be1 == 0.0))
    has_g2 = not (np.all(g2 == 1.0) and np.all(be2 == 0.0))

    key = (BC, N, DZ, DA, E, H1, H2, has_b2, has_b3, has_g1, has_g2)
    nc = _get_nc(key, *key)

    # x native, padded with ones: [B, N, XA] -> [B, 128, NT, XA]
    xf = np.empty((B, N, XA), np.float32)
    xf[:, :, 0:DZ] = z
    xf[:, :, DZ:D] = a
    xf[:, :, D:XA] = 1.0
    xa_all = np.ascontiguousarray(
        xf.reshape(B, 128, NT, XA)
    ).astype(BF)  # token n = p*NT + t
    # x transposed, tile-shuffled: [B, D, NT, 128] with col (t, p)
    xT = np.concatenate([z, a], axis=-1).transpose(0, 2, 1)  # [B, D, N]
    xTs = xT.reshape(B, D, 128, NT).transpose(0, 1, 3, 2)  # [B, D, NT, 128]
    zTs_all = np.ascontiguousarray(xTs[:, 0:128]).astype(BF)
    aTs_all = np.ascontiguousarray(xTs[:, 128:D]).astype(BF)

    phi2 = np.ascontiguousarray(phi.reshape(D, -1)).astype(BF)
    w1aug = np.concatenate([W1, b1[:, None, :]], axis=1)  # [E, D+1, H1]
    identb = np.eye(128, dtype=np.float32).astype(BF)

    in_maps = []
    for i in range(NCORES):
        es = slice(i * EL, (i + 1) * EL)
        m = {
            "xa": xa_all[i * BC : (i + 1) * BC],
            "zTs": zTs_all[i * BC : (i + 1) * BC],
            "aTs": aTs_all[i * BC : (i + 1) * BC],
            "phi_d": phi2,
            "w1h_d": np.ascontiguousarray(w1aug[es, 0:128]).astype(BF),
            "w1l_d": np.ascontiguousarray(w1aug[es, 128 : D + 1]).astype(BF),
            "w2_d": np.ascontiguousarray(
                W2[es].reshape(EL, H1 // 128, 128, H2)
            ).astype(BF),
            "w3_d": np.ascontiguousarray(
                W3[es].reshape(EL, H2 // 128, 128, DZ)
            ).astype(BF),
            "identb_d": identb,
        }
        if has_b2:
            m["b2_d"] = np.ascontiguousarray(b2[es].reshape(1, -1)).astype(BF)
            m["ones_d"] = np.ones((1, 128), np.float32).astype(BF)
        if has_b3:
            m["b3_d"] = np.ascontiguousarray(b3[es].reshape(1, -1)).astype(BF)
        if has_g1:
            m["g1_d"] = np.ascontiguousarray(np.tile(np.repeat(g1[es], 32, 0), (2, 1)))
            m["be1_d"] = np.ascontiguousarray(np.tile(np.repeat(be1[es], 32, 0), (2, 1)))
        if has_g2:
            m["g2_d"] = np.ascontiguousarray(np.tile(np.repeat(g2[es], 32, 0), (2, 1)))
            m["be2_d"] = np.ascontiguousarray(np.tile(np.repeat(be2[es], 32, 0), (2, 1)))
        in_maps.append(m)
    return nc, in_maps


def kernel(**inputs):
    nc, in_maps = _prepare(**inputs)

    from concourse.bass_utils import run_bass_kernel_spmd

    res = run_bass_kernel_spmd(nc, in_maps, list(range(NCORES)))
    B = len(in_maps) * in_maps[0]["xa"].shape[0]
    outs = []
    for r in res.results:
        o = r["out"]  # [BC, 128, NT, DZ], token n = p*NT + t
        BC, P, NT_, DZ_ = o.shape
        outs.append(o.reshape(BC, P * NT_, DZ_))
    return np.concatenate(outs, axis=0)
